# revision 18
# baseline (speedup 1.0000x reference)
"""AdaptiveGeometryAttention distributed Bass kernel for 8 trn2 NeuronCores.

Sharding: data-parallel over B (2 groups of 4 cores), head-parallel over H
(4 heads per core). Each core computes its heads' attention and a partial
out-projection [T, C]; ReduceScatter(add) over each 4-core group leaves
each core with a shard of the final output, which the host reassembles.

Attention runs in the transposed domain (scores stored [s, t]) so no
per-tile transposes are needed: softmax denominators come from a
ones-vector matmul on the TensorEngine, and per-t scalars (alpha, spike,
1/denom) are applied via gpsimd partition_broadcast tiles.

Self-contained: hardcodes all shapes; host side only shards/transposes
inputs and concatenates the output shards.
"""
import os
import sys

for _p in ("/opt/trn_rl_repo",):
    if _p not in sys.path:
        sys.path.append(_p)

import numpy as np
import concourse.bass as bass
import concourse.bacc as bacc
import concourse.mybir as mybir
from concourse import masks
from concourse.alu_op_type import AluOpType
from concourse.tile import TileContext
from concourse.bass_utils import run_bass_kernel_spmd

AF = mybir.ActivationFunctionType
DT = mybir.dt

B, T, C, H, D = 2, 1024, 1024, 16, 64
HL = 4                 # heads per core
JD = HL * D            # 256 local head dims
N_CORES = 8
GROUPS = [[0, 1, 2, 3], [4, 5, 6, 7]]
SQD = 0.125            # 1/sqrt(D)
NEG = -1.0e30
RS_SPLIT = 6           # first reduce-scatter covers t-tiles [0, RS_SPLIT)

PROJ_F32R = True
NI_F32R = True

KSTATS = {}


def build_nc():
    nc = bacc.Bacc("TRN2")
    DT_PROJ = DT.float32r if PROJ_F32R else DT.float32
    DT_NI = DT.float32r if NI_F32R else DT.float32

    # ---- I/O ----
    xT_e = nc.dram_tensor("xT", [C, T], DT.float32, kind="ExternalInput")
    wqT_e = nc.dram_tensor("wqT", [C, JD], DT_PROJ, kind="ExternalInput")
    wkT_e = nc.dram_tensor("wkT", [C, JD], DT_PROJ, kind="ExternalInput")
    wvT_e = nc.dram_tensor("wvT", [C, JD], DT_PROJ, kind="ExternalInput")
    bq_e = nc.dram_tensor("bq_b", [128, JD], DT.float32, kind="ExternalInput")
    bk_e = nc.dram_tensor("bk_b", [128, JD], DT.float32, kind="ExternalInput")
    bv_e = nc.dram_tensor("bv_b", [128, JD], DT.float32, kind="ExternalInput")
    wiaT_e = nc.dram_tensor("wiaT", [C, 5], DT.float32, kind="ExternalInput")
    bia_e = nc.dram_tensor("bia_b", [128, 5], DT.float32, kind="ExternalInput")
    woT_e = nc.dram_tensor("woT", [JD, C], DT.float32, kind="ExternalInput")
    bout_e = nc.dram_tensor("bout_b", [128, C], DT.float32, kind="ExternalInput")
    thneg_e = nc.dram_tensor("thneg_b", [128, 1], DT.float32, kind="ExternalInput")
    cmask_e = nc.dram_tensor("cmaskT", [128, 128], DT.float32, kind="ExternalInput")
    out_e = nc.dram_tensor("out", [T // 4, C], DT.float32, kind="ExternalOutput")

    partial_d = nc.dram_tensor("partial_d", [T, C], DT.float32)
    rs_out_d = nc.dram_tensor("rs_out_d", [T // 4, C], DT.float32)

    SP1 = RS_SPLIT * 128

    with TileContext(nc) as tc:
        with (
            tc.tile_pool(name="const", bufs=1) as cpool,
            tc.tile_pool(name="mainp", bufs=1) as mp,
            tc.tile_pool(name="psA", bufs=3, space="PSUM") as psA,
            tc.tile_pool(name="psY", bufs=1, space="PSUM") as psY,
            tc.tile_pool(name="psD", bufs=1, space="PSUM") as psD,
        ):
            # ---- constants ----
            idf = cpool.tile([128, 128], DT.float32, tag="idf")
            masks.make_identity(nc, idf[:])
            cmaskT = cpool.tile([128, 128], DT.float32, tag="cmaskT")
            nc.sync.dma_start(out=cmaskT[:], in_=cmask_e[:])
            bout_b = cpool.tile([128, C], DT.float32, tag="boutb")
            nc.sync.dma_start(out=bout_b[:], in_=bout_e[:])
            thneg = cpool.tile([128, 1], DT.float32, tag="thneg")
            nc.sync.dma_start(out=thneg[:], in_=thneg_e[:])
            negone = cpool.tile([128, 1], DT.float32, tag="negone")
            nc.vector.memset(negone[:], -1.0)
            onesbf = cpool.tile([128, 1], DT.bfloat16, tag="onesbf")
            nc.vector.memset(onesbf[:], 1.0)

            # ---- persistent main tiles ----
            vbf = mp.tile([128, 8 * JD], DT.bfloat16, tag="vbf")
            qbT = [mp.tile([128, T], DT.bfloat16, tag=f"qbT{j}", name=f"qbT{j}") for j in range(2)]
            kbT = [mp.tile([128, T], DT.bfloat16, tag=f"kbT{j}", name=f"kbT{j}") for j in range(2)]
            qhT = [mp.tile([128, T], DT_NI, tag=f"qhT{j}", name=f"qhT{j}") for j in range(2)]
            khT = [mp.tile([128, T], DT_NI, tag=f"khT{j}", name=f"khT{j}") for j in range(2)]
            wobf = mp.tile([128, 2 * C], DT.bfloat16, tag="wobf")
            statT = mp.tile([40, 128], DT.float32, tag="statT")
            # stat rows at partition 0: 0..31 = -alpha (ti*4+h), 32..39 = spike (ti)
            strow = [mp.tile([1, 128], DT.float32, tag=f"strow{i}", name=f"strow{i}")
                     for i in range(40)]

            with tc.tile_pool(name="wpool", bufs=1) as wp:
                # ---- loads ----
                xT = wp.tile([128, 8 * T], DT.float32, tag="xT")
                for kc in range(8):
                    nc.sync.dma_start(
                        out=xT[:, kc * T:(kc + 1) * T],
                        in_=xT_e[kc * 128:(kc + 1) * 128, :],
                    )
                wq = wp.tile([128, 8 * JD], DT_PROJ, tag="wq")
                wk = wp.tile([128, 8 * JD], DT_PROJ, tag="wk")
                wv = wp.tile([128, 8 * JD], DT_PROJ, tag="wv")
                for w_t, w_e in ((wq, wqT_e), (wk, wkT_e), (wv, wvT_e)):
                    for kc in range(8):
                        nc.sync.dma_start(
                            out=w_t[:, kc * JD:(kc + 1) * JD],
                            in_=w_e[kc * 128:(kc + 1) * 128, :],
                        )
                if PROJ_F32R:
                    xTr = wp.tile([128, 8 * T], DT.float32r, tag="xTr")
                    nc.vector.tensor_copy(xTr[:], xT[:])
                else:
                    xTr = xT
                wia = wp.tile([128, 8 * 5], DT.float32, tag="wia")
                for kc in range(8):
                    nc.sync.dma_start(
                        out=wia[:, kc * 5:(kc + 1) * 5],
                        in_=wiaT_e[kc * 128:(kc + 1) * 128, :],
                    )
                wo = wp.tile([128, 2 * C], DT.float32, tag="wo")
                for cc in range(2):
                    nc.sync.dma_start(
                        out=wo[:, cc * C:(cc + 1) * C],
                        in_=woT_e[cc * 128:(cc + 1) * 128, :],
                    )
                bq_b = wp.tile([128, JD], DT.float32, tag="bqb")
                bk_b = wp.tile([128, JD], DT.float32, tag="bkb")
                bv_b = wp.tile([128, JD], DT.float32, tag="bvb")
                bia_b = wp.tile([128, 5], DT.float32, tag="biab")
                nc.sync.dma_start(out=bq_b[:], in_=bq_e[:])
                nc.sync.dma_start(out=bk_b[:], in_=bk_e[:])
                nc.sync.dma_start(out=bv_b[:], in_=bv_e[:])
                nc.sync.dma_start(out=bia_b[:], in_=bia_e[:])

                nc.vector.tensor_copy(wobf[:], wo[:])

                # ---- phase 1: projections (row layout [t, d']) ----
                qrow = wp.tile([128, 8 * JD], DT.float32, tag="qrow")
                krow = wp.tile([128, 8 * JD], DT.float32, tag="krow")
                ia_sb = wp.tile([128, 8 * 5], DT.float32, tag="iasb")

                for t8 in range(8):
                    ps = psA.tile([128, 1024], DT.float32, tag="ps")
                    for kc in range(8):
                        nc.tensor.matmul(
                            ps[:, 0:5],
                            xT[:, kc * T + t8 * 128: kc * T + t8 * 128 + 128],
                            wia[:, kc * 5:(kc + 1) * 5],
                            start=(kc == 0), stop=(kc == 7),
                        )
                    tmp5 = wp.tile([128, 5], DT.float32, tag="tmp5")
                    nc.vector.tensor_add(tmp5[:], ps[:, 0:5], bia_b[:])
                    nc.scalar.activation(ia_sb[:, t8 * 5:(t8 + 1) * 5], tmp5[:], AF.Sigmoid)

                for dst, w_t, b_t in (
                    (qrow, wq, bq_b),
                    (krow, wk, bk_b),
                    (vbf, wv, bv_b),
                ):
                    for t8 in range(8):
                        ps = psA.tile([128, 1024], DT.float32, tag="ps")
                        for kc in range(8):
                            nc.tensor.matmul(
                                ps[:, 0:JD],
                                xTr[:, kc * T + t8 * 128: kc * T + t8 * 128 + 128],
                                w_t[:, kc * JD:(kc + 1) * JD],
                                start=(kc == 0), stop=(kc == 7),
                            )
                        nc.vector.tensor_add(
                            dst[:, t8 * JD:(t8 + 1) * JD], ps[:, 0:JD], b_t[:]
                        )

                # ---- spike / alpha stats ----
                stat40 = wp.tile([128, 40], DT.float32, tag="stat40")
                ia3 = ia_sb[:].rearrange("p (t f) -> p t f", f=5)
                nc.vector.memset(ia_sb[0:1, 0:1], 0.0)   # cmask t=0
                imp8 = wp.tile([128, 8], DT.float32, tag="imp8")
                nc.vector.tensor_scalar(imp8[:], ia3[:, :, 0:1], thneg[:], None, AluOpType.add)
                sgn8 = wp.tile([128, 8], DT.float32, tag="sgn8")
                nc.scalar.activation(sgn8[:], imp8[:], AF.Sign)
                nc.vector.tensor_scalar_max(stat40[:, 32:40], sgn8[:], 0.0)
                nc.vector.tensor_scalar_mul(
                    stat40[:, 0:32].rearrange("p (t f) -> p t f", f=4),
                    ia3[:, :, 1:5], -1.0,
                )
                oma = wp.tile([128, 32], DT.float32, tag="oma")
                nc.vector.tensor_scalar(
                    oma[:].rearrange("p (t f) -> p t f", f=4), ia3[:, :, 1:5], -1.0,
                    1.0, AluOpType.mult, AluOpType.add,
                )
                pst = psA.tile([128, 1024], DT.float32, tag="ps")
                nc.tensor.transpose(pst[0:40, 0:128], stat40[:], idf[:])
                nc.scalar.copy(statT[:], pst[0:40, 0:128])
                for i in range(40):
                    nc.sync.dma_start(out=strow[i][:], in_=statT[i:i + 1, :])

                # ---- phase 2: row stats + modified rows ----
                sqq = wp.tile([128, 8 * JD], DT.float32, tag="big5", bufs=3)
                sqk = wp.tile([128, 8 * JD], DT.float32, tag="big5", bufs=3)
                nc.scalar.activation(sqq[:], qrow[:], AF.Square)
                nc.scalar.activation(sqk[:], krow[:], AF.Square)
                n2 = wp.tile([128, 64], DT.float32, tag="n2")
                for side, sq_t in ((0, sqq), (1, sqk)):
                    for t8 in range(8):
                        for h in range(HL):
                            col = side * 32 + t8 * 4 + h
                            nc.vector.tensor_reduce(
                                n2[:, col:col + 1],
                                sq_t[:, t8 * JD + h * D + 1: t8 * JD + (h + 1) * D],
                                mybir.AxisListType.X, AluOpType.add,
                            )
                lnn = wp.tile([128, 64], DT.float32, tag="lnn")
                nc.scalar.activation(lnn[:], n2[:], AF.Ln)
                nrm = wp.tile([128, 64], DT.float32, tag="nrm")
                nc.scalar.activation(nrm[:], lnn[:], AF.Exp, scale=0.5)
                nc.vector.tensor_scalar_max(nrm[:], nrm[:], 1e-7)
                e1 = wp.tile([128, 64], DT.float32, tag="e1")
                e2 = wp.tile([128, 64], DT.float32, tag="e2")
                nc.scalar.activation(e1[:], nrm[:], AF.Exp)
                nc.scalar.activation(e2[:], nrm[:], AF.Exp, scale=-1.0)
                csh = wp.tile([128, 64], DT.float32, tag="csh")
                nc.vector.tensor_add(csh[:], e1[:], e2[:])
                nc.vector.tensor_scalar_mul(csh[:], csh[:], 0.5)
                snh = wp.tile([128, 64], DT.float32, tag="snh")
                nc.vector.tensor_sub(snh[:], e1[:], e2[:])
                rcn = wp.tile([128, 64], DT.float32, tag="rcn")
                nc.vector.reciprocal(rcn[:], nrm[:])
                rat = wp.tile([128, 64], DT.float32, tag="rat")
                nc.vector.scalar_tensor_tensor(
                    rat[:], snh[:], 0.5, rcn[:], AluOpType.mult, AluOpType.mult
                )

                # qb2 = (1-alpha)*q ; qhat = [coshq | ratq*q] ; khat = [coshk | -ratk*k]
                qb2 = wp.tile([128, 8 * JD], DT.float32, tag="big5", bufs=3)
                qhat = wp.tile([128, 8 * JD], DT.float32, tag="big5", bufs=3)
                khat = wp.tile([128, 8 * JD], DT.float32, tag="big5", bufs=3)
                nrk = wp.tile([128, 32], DT.float32, tag="nrk")
                nc.vector.tensor_scalar_mul(nrk[:], rat[:, 32:64], -1.0)
                for t8 in range(8):
                    for h in range(HL):
                        col = t8 * 4 + h
                        base = t8 * JD + h * D
                        nc.vector.tensor_scalar_mul(
                            qb2[:, base:base + D], qrow[:, base:base + D],
                            oma[:, col:col + 1],
                        )
                        nc.vector.tensor_scalar_mul(
                            qhat[:, base + 1:base + D], qrow[:, base + 1:base + D],
                            rat[:, col:col + 1],
                        )
                        nc.vector.tensor_copy(qhat[:, base:base + 1], csh[:, col:col + 1])
                        nc.vector.tensor_scalar_mul(
                            khat[:, base + 1:base + D], krow[:, base + 1:base + D],
                            nrk[:, col:col + 1],
                        )
                        nc.vector.tensor_copy(khat[:, base:base + 1], csh[:, 32 + col:33 + col])

                # transposes: row layout [t, c'] -> column layout [c', t]
                for src, dsts in (
                    (qb2, qbT),
                    (krow, kbT),
                    (qhat, qhT),
                    (khat, khT),
                ):
                    for jc in range(2):
                        pst = psA.tile([128, 1024], DT.float32, tag="ps")
                        for t8 in range(8):
                            nc.tensor.transpose(
                                pst[:, t8 * 128:(t8 + 1) * 128],
                                src[:, t8 * JD + jc * 128: t8 * JD + (jc + 1) * 128],
                                idf[:],
                            )
                        nc.scalar.copy(dsts[jc][:], pst[:])

            # ---- phase 3: attention + out-projection (transposed domain) ----
            with tc.tile_pool(name="pipe", bufs=2) as pp, \
                 tc.tile_pool(name="pipeb", bufs=4) as pb_pool, \
                 tc.tile_pool(name="pipeo", bufs=2) as po, \
                 tc.tile_pool(name="pipes", bufs=4) as sp:
                for ti in range(8):
                    S = (ti + 1) * 128
                    psy = psY.tile([128, 2 * 128], DT.float32, tag="psy")
                    pbts = {}
                    for pr in range(2):
                        A = pp.tile([128, 2048], DT.float32, tag="A", bufs=3)
                        Bt = pp.tile([128, 2048], DT.float32, tag="B", bufs=3)
                        Ct = pp.tile([128, 2048], DT.float32, tag="C", bufs=3)
                        for hh in range(2):
                            h = pr * 2 + hh
                            jc = h // 2
                            hb = hh * S
                            se = psA.tile([128, 1024], DT.float32, tag="ps")
                            ni = psA.tile([128, 1024], DT.float32, tag="ps")
                            for sj in range(ti + 1):
                                nc.tensor.matmul(
                                    se[:, sj * 128:(sj + 1) * 128],
                                    kbT[jc][hh * 64:(hh + 1) * 64, sj * 128:(sj + 1) * 128],
                                    qbT[jc][hh * 64:(hh + 1) * 64, ti * 128:(ti + 1) * 128],
                                    start=True, stop=True,
                                )
                                nc.tensor.matmul(
                                    ni[:, sj * 128:(sj + 1) * 128],
                                    khT[jc][hh * 64:(hh + 1) * 64, sj * 128:(sj + 1) * 128],
                                    qhT[jc][hh * 64:(hh + 1) * 64, ti * 128:(ti + 1) * 128],
                                    start=True, stop=True,
                                )
                            # A_h = se' (+ causal mask on the diagonal chunk)
                            if ti > 0:
                                nc.scalar.copy(A[:, hb:hb + ti * 128], se[:, :ti * 128])
                            nc.vector.tensor_add(
                                A[:, hb + ti * 128:hb + S], se[:, ti * 128:S], cmaskT[:]
                            )
                            nc.vector.tensor_scalar_max(
                                Bt[:, hb:hb + S], ni[:, :S], 1.0 + 1e-7
                            )
                        W2 = 2 * S
                        nc.gpsimd.tensor_mul(Ct[:, :W2], Bt[:, :W2], Bt[:, :W2])
                        nc.scalar.activation(Ct[:, :W2], Ct[:, :W2], AF.Ln, bias=negone[:])
                        nc.scalar.activation(Ct[:, :W2], Ct[:, :W2], AF.Exp, scale=0.5)
                        nc.gpsimd.tensor_add(Bt[:, :W2], Bt[:, :W2], Ct[:, :W2])
                        nc.scalar.activation(Bt[:, :W2], Bt[:, :W2], AF.Ln)
                        nc.gpsimd.tensor_mul(Bt[:, :W2], Bt[:, :W2], Bt[:, :W2])
                        for hh in range(2):
                            h = pr * 2 + hh
                            col = ti * 4 + h
                            hb = hh * S
                            nab = sp.tile([128, 128], DT.float32, tag="nab")
                            nc.gpsimd.partition_broadcast(nab[:], strow[col][:])
                            nbc = nab[:].unsqueeze(1).broadcast_to((128, ti + 1, 128))
                            nc.vector.tensor_mul(
                                Bt[:, hb:hb + S].rearrange("p (b c) -> p b c", c=128),
                                Bt[:, hb:hb + S].rearrange("p (b c) -> p b c", c=128),
                                nbc,
                            )
                            nc.vector.tensor_add(
                                Bt[:, hb:hb + S], Bt[:, hb:hb + S], A[:, hb:hb + S]
                            )
                            nc.scalar.activation(A[:, hb:hb + S], Bt[:, hb:hb + S],
                                                 AF.Exp, scale=SQD)
                            pbr = pb_pool.tile([128, 1024], DT.bfloat16, tag="pbr")
                            nc.vector.tensor_copy(pbr[:, :S], A[:, hb:hb + S])
                            dps = psD.tile([1, 128], DT.float32, tag="dps")
                            for sj in range(ti + 1):
                                nc.tensor.matmul(
                                    dps[:],
                                    onesbf[:, 0:1],
                                    pbr[:, sj * 128:(sj + 1) * 128],
                                    start=(sj == 0), stop=(sj == ti),
                                )
                            rrow = sp.tile([1, 128], DT.float32, tag="rrow")
                            nc.vector.reciprocal(rrow[:], dps[:])
                            srow = sp.tile([1, 128], DT.float32, tag="srow")
                            nc.vector.tensor_mul(srow[:], rrow[:], strow[32 + ti][:])
                            scb = sp.tile([128, 128], DT.float32, tag="scb")
                            nc.gpsimd.partition_broadcast(scb[:], srow[:])
                            pbt = pb_pool.tile([128, 1024], DT.bfloat16, tag="pbt")
                            nc.vector.tensor_mul(
                                pbt[:, :S].rearrange("p (b c) -> p b c", c=128),
                                pbr[:, :S].rearrange("p (b c) -> p b c", c=128),
                                scb[:].unsqueeze(1).broadcast_to((128, ti + 1, 128)),
                            )
                            pbts[h] = pbt
                    for h in range(HL):
                        jc, hh = h // 2, h % 2
                        for sj in range(ti + 1):
                            nc.tensor.matmul(
                                psy[hh * 64:(hh + 1) * 64, jc * 128:(jc + 1) * 128],
                                vbf[:, sj * JD + h * D: sj * JD + (h + 1) * D],
                                pbts[h][:, sj * 128:(sj + 1) * 128],
                                start=(sj == 0), stop=(sj == ti),
                                tile_position=(0, hh * 64),
                            )
                    # out projection for this t-tile
                    yT0 = sp.tile([128, 128], DT.bfloat16, tag="yT0")
                    yT1 = sp.tile([128, 128], DT.bfloat16, tag="yT1")
                    nc.vector.tensor_copy(yT0[:], psy[:, 0:128])
                    nc.vector.tensor_copy(yT1[:], psy[:, 128:256])
                    out_sb = po.tile([128, 1024], DT.float32, tag="outsb")
                    for oc in range(2):
                        pso = psA.tile([128, 1024], DT.float32, tag="ps")
                        for cc, yT_t in ((0, yT0), (1, yT1)):
                            nc.tensor.matmul(
                                pso[:, 0:512],
                                yT_t[:],
                                wobf[:, cc * C + oc * 512: cc * C + oc * 512 + 512],
                                start=(cc == 0), stop=(cc == 1),
                            )
                        nc.vector.tensor_copy(out_sb[:, oc * 512:(oc + 1) * 512], pso[:, 0:512])
                    nc.sync.dma_start(
                        out=partial_d[ti * 128:(ti + 1) * 128, :], in_=out_sb[:]
                    )
                    if ti == RS_SPLIT - 1:
                        nc.gpsimd.collective_compute(
                            "ReduceScatter", mybir.AluOpType.add,
                            replica_groups=GROUPS,
                            ins=[partial_d[0:SP1, :]],
                            outs=[rs_out_d[0:SP1 // 4, :]],
                        )

                nc.gpsimd.collective_compute(
                    "ReduceScatter", mybir.AluOpType.add,
                    replica_groups=GROUPS,
                    ins=[partial_d[SP1:T, :]],
                    outs=[rs_out_d[SP1 // 4:T // 4, :]],
                )
                for r in range(2):
                    fin = po.tile([128, 1024], DT.float32, tag="fin")
                    nc.sync.dma_start(out=fin[:], in_=rs_out_d[r * 128:(r + 1) * 128, :])
                    nc.vector.tensor_add(fin[:], fin[:], bout_b[:])
                    nc.sync.dma_start(out=out_e[r * 128:(r + 1) * 128, :], in_=fin[:])

    nc.finalize()
    return nc


_NC = None


def _get_nc():
    global _NC
    if _NC is None:
        _NC = build_nc()
    return _NC


def _shard_inputs(inputs):
    x = np.asarray(inputs["x"], np.float32)
    Wqkv = np.asarray(inputs["Wqkv"], np.float32)
    bqkv = np.asarray(inputs["bqkv"], np.float32)
    Wout = np.asarray(inputs["Wout"], np.float32)
    bout = np.asarray(inputs["bout"], np.float32)
    Wimp = np.asarray(inputs["Wimp"], np.float32)
    bimp = np.asarray(inputs["bimp"], np.float32)
    Walpha = np.asarray(inputs["Walpha"], np.float32)
    balpha = np.asarray(inputs["balpha"], np.float32)
    th = np.asarray(inputs["threshold"], np.float32)

    cmaskT = np.tril(np.full((128, 128), NEG, np.float32), -1)
    in_maps = []
    for core in range(N_CORES):
        b = core // 4
        hs = (core % 4) * HL
        sl = slice(hs * D, (hs + HL) * D)
        m = {
            "xT": np.ascontiguousarray(x[b].T),
            "wqT": np.ascontiguousarray(Wqkv[sl].T),
            "wkT": np.ascontiguousarray(Wqkv[C + hs * D: C + (hs + HL) * D].T),
            "wvT": np.ascontiguousarray(Wqkv[2 * C + hs * D: 2 * C + (hs + HL) * D].T),
            "bq_b": np.ascontiguousarray(np.broadcast_to(bqkv[sl], (128, JD))),
            "bk_b": np.ascontiguousarray(
                np.broadcast_to(bqkv[C + hs * D: C + (hs + HL) * D], (128, JD))),
            "bv_b": np.ascontiguousarray(
                np.broadcast_to(bqkv[2 * C + hs * D: 2 * C + (hs + HL) * D], (128, JD))),
            "wiaT": np.ascontiguousarray(
                np.concatenate([Wimp, Walpha[hs:hs + HL]], 0).T),
            "bia_b": np.ascontiguousarray(np.broadcast_to(
                np.concatenate([bimp, balpha[hs:hs + HL]]), (128, 5))),
            "woT": np.ascontiguousarray(Wout[:, sl].T),
            "bout_b": np.ascontiguousarray(np.broadcast_to(bout, (128, C))),
            "thneg_b": np.full((128, 1), -th[0], np.float32),
            "cmaskT": cmaskT,
        }
        in_maps.append(m)
    return in_maps


def _assemble(results):
    q1 = RS_SPLIT * 32          # rows per rank from RS#1 (192 for split 6)
    out = np.zeros((B, T, C), np.float32)
    for core in range(N_CORES):
        b, r = core // 4, core % 4
        o = results[core]["out"]
        out[b, r * q1:(r + 1) * q1, :] = o[0:q1]
        q2 = 256 - q1
        base = RS_SPLIT * 128
        out[b, base + r * q2: base + (r + 1) * q2, :] = o[q1:256]
    return out


def kernel(**inputs):
    nc = _get_nc()
    in_maps = _shard_inputs(inputs)
    trace = os.environ.get("KERNEL_PROFILE", "") == "1"
    res = run_bass_kernel_spmd(
        nc, in_maps, core_ids=list(range(N_CORES)), trace=trace
    )
    KSTATS["exec_time_ns"] = res.exec_time_ns
    return _assemble(res.results)


# revision 19
# speedup vs baseline: 1.3099x; 1.3099x over previous
"""AdaptiveGeometryAttention distributed Bass kernel for 8 trn2 NeuronCores.

Sharding: data-parallel over B (2 groups of 4 cores), head-parallel over H
(4 heads per core). Each core computes its heads' attention and a partial
out-projection [T, C]; a ReduceScatter(add) over each 4-core group leaves
each core with a distinct 256-row shard of the final output, which the host
reassembles.

Self-contained: hardcodes all shapes; host side only shards/transposes
inputs and concatenates the output shards.
"""
import os
import sys

for _p in ("/opt/trn_rl_repo",):
    if _p not in sys.path:
        sys.path.append(_p)

import numpy as np
import concourse.bass as bass
import concourse.bacc as bacc
import concourse.mybir as mybir
from concourse import masks
from concourse.alu_op_type import AluOpType
from concourse.tile import TileContext
from concourse.bass_utils import run_bass_kernel_spmd

AF = mybir.ActivationFunctionType
DT = mybir.dt

B, T, C, H, D = 2, 1024, 1024, 16, 64
HL = 4                 # heads per core
JD = HL * D            # 256 local head dims
N_CORES = 8
GROUPS = [[0, 1, 2, 3], [4, 5, 6, 7]]
SQD = 0.125            # 1/sqrt(D)
NEG = -1.0e30

# dtype knobs
PROJ_F32R = True       # q/k/v/ia projection matmuls via float32r operands
NI_F32R = True         # neg_inner matmul via float32r operands

KSTATS = {}


def _f32r(ap):
    return ap.bitcast(DT.float32r)


def _mmdt(ap, use_f32r):
    return _f32r(ap) if use_f32r else ap


def build_nc():
    nc = bacc.Bacc("TRN2")

    # ---- I/O ----
    xT_e = nc.dram_tensor("xT", [C, T], DT.float32, kind="ExternalInput")
    DT_PROJ = DT.float32r if PROJ_F32R else DT.float32
    DT_NI = DT.float32r if NI_F32R else DT.float32
    wqT_e = nc.dram_tensor("wqT", [C, JD], DT_PROJ, kind="ExternalInput")
    wkT_e = nc.dram_tensor("wkT", [C, JD], DT_PROJ, kind="ExternalInput")
    wvT_e = nc.dram_tensor("wvT", [C, JD], DT_PROJ, kind="ExternalInput")
    bq_e = nc.dram_tensor("bq_b", [128, JD], DT.float32, kind="ExternalInput")
    bk_e = nc.dram_tensor("bk_b", [128, JD], DT.float32, kind="ExternalInput")
    bv_e = nc.dram_tensor("bv_b", [128, JD], DT.float32, kind="ExternalInput")
    wiaT_e = nc.dram_tensor("wiaT", [C, 5], DT.float32, kind="ExternalInput")
    bia_e = nc.dram_tensor("bia_b", [128, 5], DT.float32, kind="ExternalInput")
    woT_e = nc.dram_tensor("woT", [JD, C], DT.float32, kind="ExternalInput")
    bout_e = nc.dram_tensor("bout_b", [128, C], DT.float32, kind="ExternalInput")
    thneg_e = nc.dram_tensor("thneg_b", [128, 1], DT.float32, kind="ExternalInput")
    cmask_e = nc.dram_tensor("cmask", [128, 128], DT.float32, kind="ExternalInput")
    out_e = nc.dram_tensor("out", [T // 4, C], DT.float32, kind="ExternalOutput")

    partial1_d = nc.dram_tensor("partial1_d", [768, C], DT.float32)
    partial2_d = nc.dram_tensor("partial2_d", [256, C], DT.float32)
    rs1_d = nc.dram_tensor("rs1_d", [192, C], DT.float32)
    rs2_d = nc.dram_tensor("rs2_d", [64, C], DT.float32)

    with TileContext(nc) as tc:
        with (
            tc.tile_pool(name="const", bufs=1) as cpool,
            tc.tile_pool(name="mainp", bufs=1) as mp,
            tc.tile_pool(name="psA", bufs=3, space="PSUM") as psA,
            tc.tile_pool(name="psY", bufs=1, space="PSUM") as psY,
            tc.tile_pool(name="psT", bufs=1, space="PSUM") as psT,
        ):
            # ---- constants ----
            idf = cpool.tile([128, 128], DT.float32, tag="idf")
            masks.make_identity(nc, idf[:])
            idbf = cpool.tile([128, 128], DT.bfloat16, tag="idbf")
            masks.make_identity(nc, idbf[:])
            cmask = cpool.tile([128, 128], DT.float32, tag="cmask")
            nc.sync.dma_start(out=cmask[:], in_=cmask_e[:])
            bout_b = cpool.tile([128, C], DT.float32, tag="boutb")
            nc.sync.dma_start(out=bout_b[:], in_=bout_e[:])
            thneg = cpool.tile([128, 1], DT.float32, tag="thneg")
            nc.sync.dma_start(out=thneg[:], in_=thneg_e[:])
            negone = cpool.tile([128, 1], DT.float32, tag="negone")
            nc.vector.memset(negone[:], -1.0)

            # ---- persistent main tiles ----
            vbf = mp.tile([128, 8 * JD], DT.bfloat16, tag="vbf")
            qbT = [mp.tile([128, T], DT.bfloat16, tag=f"qbT{j}", name=f"qbT{j}") for j in range(2)]
            kbT = [mp.tile([128, T], DT.bfloat16, tag=f"kbT{j}", name=f"kbT{j}") for j in range(2)]
            qhT = [mp.tile([128, T], DT_NI, tag=f"qhT{j}", name=f"qhT{j}") for j in range(2)]
            khT = [mp.tile([128, T], DT_NI, tag=f"khT{j}", name=f"khT{j}") for j in range(2)]
            wobf = mp.tile([128, 2 * C], DT.bfloat16, tag="wobf")
            nalpha = mp.tile([128, 32], DT.float32, tag="nalpha")
            oma = mp.tile([128, 32], DT.float32, tag="oma")
            spike = mp.tile([128, 8], DT.float32, tag="spike")
            # row stats, col = side*32 + ti*4 + h
            rat = mp.tile([128, 64], DT.float32, tag="rat")    # sinh(n)/n
            gq = mp.tile([128, 32], DT.float32, tag="gq")      # cosh/ratio (q side)
            coshk = mp.tile([128, 32], DT.float32, tag="coshk")
            nrk = mp.tile([128, 32], DT.float32, tag="nrk")    # -ratio_k

            with tc.tile_pool(name="wpool", bufs=1) as wp:
                # ---- load weights/activations ----
                xT = wp.tile([128, 8 * T], DT.float32, tag="xT")
                for kc in range(8):
                    nc.sync.dma_start(
                        out=xT[:, kc * T:(kc + 1) * T],
                        in_=xT_e[kc * 128:(kc + 1) * 128, :],
                    )
                wq = wp.tile([128, 8 * JD], DT_PROJ, tag="wq")
                wk = wp.tile([128, 8 * JD], DT_PROJ, tag="wk")
                wv = wp.tile([128, 8 * JD], DT_PROJ, tag="wv")
                for w_t, w_e in ((wq, wqT_e), (wk, wkT_e), (wv, wvT_e)):
                    for kc in range(8):
                        nc.sync.dma_start(
                            out=w_t[:, kc * JD:(kc + 1) * JD],
                            in_=w_e[kc * 128:(kc + 1) * 128, :],
                        )
                if PROJ_F32R:
                    xTr = wp.tile([128, 8 * T], DT.float32r, tag="xTr")
                    nc.vector.tensor_copy(xTr[:], xT[:])
                else:
                    xTr = xT
                wia = wp.tile([128, 8 * 5], DT.float32, tag="wia")
                for kc in range(8):
                    nc.sync.dma_start(
                        out=wia[:, kc * 5:(kc + 1) * 5],
                        in_=wiaT_e[kc * 128:(kc + 1) * 128, :],
                    )
                wo = wp.tile([128, 2 * C], DT.float32, tag="wo")
                for cc in range(2):
                    nc.sync.dma_start(
                        out=wo[:, cc * C:(cc + 1) * C],
                        in_=woT_e[cc * 128:(cc + 1) * 128, :],
                    )
                bq_b = wp.tile([128, JD], DT.float32, tag="bqb")
                bk_b = wp.tile([128, JD], DT.float32, tag="bkb")
                bv_b = wp.tile([128, JD], DT.float32, tag="bvb")
                bia_b = wp.tile([128, 5], DT.float32, tag="biab")
                nc.sync.dma_start(out=bq_b[:], in_=bq_e[:])
                nc.sync.dma_start(out=bk_b[:], in_=bk_e[:])
                nc.sync.dma_start(out=bv_b[:], in_=bv_e[:])
                nc.sync.dma_start(out=bia_b[:], in_=bia_e[:])

                nc.vector.tensor_copy(wobf[:], wo[:])

                # ---- phase 1: projections (row layout [t, d']) ----
                qrow = wp.tile([128, 8 * JD], DT.float32, tag="qrow")
                krow = wp.tile([128, 8 * JD], DT.float32, tag="krow")
                ia_sb = wp.tile([128, 8 * 5], DT.float32, tag="iasb")

                # ia first so its sigmoids run before ln/exp table set loads
                for t8 in range(8):
                    ps = psA.tile([128, 1024], DT.float32, tag="ps")
                    for kc in range(8):
                        nc.tensor.matmul(
                            ps[:, 0:5],
                            xT[:, kc * T + t8 * 128: kc * T + t8 * 128 + 128],
                            wia[:, kc * 5:(kc + 1) * 5],
                            start=(kc == 0), stop=(kc == 7),
                        )
                    tmp5 = wp.tile([128, 5], DT.float32, tag="tmp5")
                    nc.vector.tensor_add(tmp5[:], ps[:, 0:5], bia_b[:])
                    nc.scalar.activation(ia_sb[:, t8 * 5:(t8 + 1) * 5], tmp5[:], AF.Sigmoid)

                for dst, w_t, b_t, outdt in (
                    (qrow, wq, bq_b, None),
                    (krow, wk, bk_b, None),
                    (vbf, wv, bv_b, DT.bfloat16),
                ):
                    for t8 in range(8):
                        ps = psA.tile([128, 1024], DT.float32, tag="ps")
                        for kc in range(8):
                            nc.tensor.matmul(
                                ps[:, 0:JD],
                                xTr[:, kc * T + t8 * 128: kc * T + t8 * 128 + 128],
                                w_t[:, kc * JD:(kc + 1) * JD],
                                start=(kc == 0), stop=(kc == 7),
                            )
                        nc.vector.tensor_add(
                            dst[:, t8 * JD:(t8 + 1) * JD], ps[:, 0:JD], b_t[:]
                        )

                # ---- spike / nalpha ----
                ia3 = ia_sb[:].rearrange("p (t f) -> p t f", f=5)
                # importance[t=0] := 0 (cmask) before threshold compare
                nc.vector.memset(ia_sb[0:1, 0:1], 0.0)
                imp8 = wp.tile([128, 8], DT.float32, tag="imp8")
                nc.vector.tensor_scalar(imp8[:], ia3[:, :, 0:1], thneg[:], None, AluOpType.add)
                sgn8 = wp.tile([128, 8], DT.float32, tag="sgn8")
                nc.scalar.activation(sgn8[:], imp8[:], AF.Sign)
                nc.vector.tensor_scalar_max(spike[:], sgn8[:], 0.0)
                nc.vector.tensor_scalar_mul(
                    nalpha[:].rearrange("p (t f) -> p t f", f=4), ia3[:, :, 1:5], -1.0
                )
                nc.vector.tensor_scalar(
                    oma[:].rearrange("p (t f) -> p t f", f=4), ia3[:, :, 1:5], -1.0,
                    1.0, AluOpType.mult, AluOpType.add,
                )

                # ---- phase 2: row stats + modified rows + transposes ----
                sqq = wp.tile([128, 8 * JD], DT.float32, tag="sqq")
                sqk = wp.tile([128, 8 * JD], DT.float32, tag="sqk")
                nc.scalar.activation(sqq[:], qrow[:], AF.Square)
                nc.scalar.activation(sqk[:], krow[:], AF.Square)
                n2 = wp.tile([128, 64], DT.float32, tag="n2")
                for side, sq_t in ((0, sqq), (1, sqk)):
                    for t8 in range(8):
                        for h in range(HL):
                            col = side * 32 + t8 * 4 + h
                            nc.vector.tensor_reduce(
                                n2[:, col:col + 1],
                                sq_t[:, t8 * JD + h * D + 1: t8 * JD + (h + 1) * D],
                                mybir.AxisListType.X, AluOpType.add,
                            )
                # n = max(exp(0.5*ln(n2)), 1e-7)
                lnn = wp.tile([128, 64], DT.float32, tag="lnn")
                nc.scalar.activation(lnn[:], n2[:], AF.Ln)
                nrm = wp.tile([128, 64], DT.float32, tag="nrm")
                nc.scalar.activation(nrm[:], lnn[:], AF.Exp, scale=0.5)
                nc.vector.tensor_scalar_max(nrm[:], nrm[:], 1e-7)
                e1 = wp.tile([128, 64], DT.float32, tag="e1")
                e2 = wp.tile([128, 64], DT.float32, tag="e2")
                nc.scalar.activation(e1[:], nrm[:], AF.Exp)
                nc.scalar.activation(e2[:], nrm[:], AF.Exp, scale=-1.0)
                csh = wp.tile([128, 64], DT.float32, tag="csh")
                nc.vector.tensor_add(csh[:], e1[:], e2[:])
                nc.vector.tensor_scalar_mul(csh[:], csh[:], 0.5)
                snh = wp.tile([128, 64], DT.float32, tag="snh")
                nc.vector.tensor_sub(snh[:], e1[:], e2[:])
                rcn = wp.tile([128, 64], DT.float32, tag="rcn")
                nc.vector.reciprocal(rcn[:], nrm[:])
                nc.vector.scalar_tensor_tensor(
                    rat[:], snh[:], 0.5, rcn[:], AluOpType.mult, AluOpType.mult
                )
                rrat = wp.tile([128, 64], DT.float32, tag="rrat")
                nc.vector.reciprocal(rrat[:], rat[:])
                nc.vector.tensor_mul(gq[:], csh[:, 0:32], rrat[:, 0:32])
                nc.vector.tensor_copy(coshk[:], csh[:, 32:64])
                nc.vector.tensor_scalar_mul(nrk[:], rat[:, 32:64], -1.0)

                # modified rows: qhat = qrow with col0 := gq; khat = -ratk*krow, col0 := coshk
                qhat = wp.tile([128, 8 * JD], DT.float32, tag="qhat")
                khat = wp.tile([128, 8 * JD], DT.float32, tag="khat")
                nc.vector.tensor_copy(qhat[:], qrow[:])
                for t8 in range(8):
                    for h in range(HL):
                        col = t8 * 4 + h
                        base = t8 * JD + h * D
                        nc.vector.tensor_copy(qhat[:, base:base + 1], gq[:, col:col + 1])
                        nc.vector.tensor_scalar_mul(
                            khat[:, base + 1: base + D],
                            krow[:, base + 1: base + D],
                            nrk[:, col:col + 1],
                        )
                        nc.vector.tensor_copy(khat[:, base:base + 1], coshk[:, col:col + 1])

                # transposes: row layout [t, c'] -> column layout [c', t]
                for src, dsts, dt_, ident in (
                    (qrow, qbT, DT.bfloat16, idf),
                    (krow, kbT, DT.bfloat16, idf),
                    (qhat, qhT, DT.float32, idf),
                    (khat, khT, DT.float32, idf),
                ):
                    for jc in range(2):
                        pst = psA.tile([128, 1024], DT.float32, tag="ps")
                        for t8 in range(8):
                            nc.tensor.transpose(
                                pst[:, t8 * 128:(t8 + 1) * 128],
                                src[:, t8 * JD + jc * 128: t8 * JD + (jc + 1) * 128],
                                ident[:],
                            )
                        nc.scalar.copy(dsts[jc][:], pst[:])

            # ---- phase 3: attention + out-projection ----
            with tc.tile_pool(name="pipe", bufs=2) as pp, \
                 tc.tile_pool(name="pipeb", bufs=3) as pb_pool, \
                 tc.tile_pool(name="pipeo", bufs=2) as po, \
                 tc.tile_pool(name="pipes", bufs=4) as sp:
                for ti in range(8):
                    S = (ti + 1) * 128
                    psy = psY.tile([128, 2 * 128], DT.float32, tag="psy")
                    pbts = {}
                    for pr in range(2):            # head pairs (0,1), (2,3)
                        A = pp.tile([128, 2048], DT.float32, tag="A", bufs=3)
                        Bt = pp.tile([128, 2048], DT.float32, tag="B", bufs=3)
                        Ct = pp.tile([128, 2048], DT.float32, tag="C", bufs=3)
                        for hh in range(2):
                            h = pr * 2 + hh
                            jc = h // 2
                            col = ti * 4 + h
                            se = psA.tile([128, 1024], DT.float32, tag="ps")
                            ni = psA.tile([128, 1024], DT.float32, tag="ps")
                            for c0 in range(0, S, 512):
                                n_sc = min(512, S - c0)
                                nc.tensor.matmul(
                                    se[:, c0:c0 + n_sc],
                                    qbT[jc][hh * 64:(hh + 1) * 64, ti * 128:(ti + 1) * 128],
                                    kbT[jc][hh * 64:(hh + 1) * 64, c0:c0 + n_sc],
                                    start=True, stop=True,
                                )
                                nc.tensor.matmul(
                                    ni[:, c0:c0 + n_sc],
                                    qhT[jc][hh * 64:(hh + 1) * 64, ti * 128:(ti + 1) * 128],
                                    khT[jc][hh * 64:(hh + 1) * 64, c0:c0 + n_sc],
                                    start=True, stop=True,
                                )
                            # A_h = (1-alpha)*se (+ cmask on diagonal block)
                            hb = hh * S
                            if ti > 0:
                                nc.vector.tensor_scalar(
                                    A[:, hb:hb + ti * 128], se[:, :ti * 128],
                                    oma[:, col:col + 1], None, AluOpType.mult,
                                )
                            nc.vector.scalar_tensor_tensor(
                                A[:, hb + ti * 128:hb + S], se[:, ti * 128:S],
                                oma[:, col:col + 1], cmask[:],
                                AluOpType.mult, AluOpType.add,
                            )
                            # B_h = max(ratio_q * ni_raw, 1+1e-7)
                            nc.vector.tensor_scalar(
                                Bt[:, hb:hb + S], ni[:, :S], rat[:, col:col + 1],
                                1.0 + 1e-7, AluOpType.mult, AluOpType.max,
                            )
                        W2 = 2 * S
                        # C = B^2 ; C = ln(C-1) ; C = exp(C/2) ; B = B + C ; B = ln(B)
                        nc.gpsimd.tensor_mul(Ct[:, :W2], Bt[:, :W2], Bt[:, :W2])
                        nc.scalar.activation(Ct[:, :W2], Ct[:, :W2], AF.Ln, bias=negone[:])
                        nc.scalar.activation(Ct[:, :W2], Ct[:, :W2], AF.Exp, scale=0.5)
                        nc.gpsimd.tensor_add(Bt[:, :W2], Bt[:, :W2], Ct[:, :W2])
                        nc.scalar.activation(Bt[:, :W2], Bt[:, :W2], AF.Ln)
                        # B = B^2 (= d^2)
                        nc.gpsimd.tensor_mul(Bt[:, :W2], Bt[:, :W2], Bt[:, :W2])
                        for hh in range(2):
                            h = pr * 2 + hh
                            jc = h // 2
                            col = ti * 4 + h
                            hb = hh * S
                            # z = -alpha*d^2 + (1-alpha)*se   (in place into B)
                            nc.vector.scalar_tensor_tensor(
                                Bt[:, hb:hb + S], Bt[:, hb:hb + S],
                                nalpha[:, col:col + 1], A[:, hb:hb + S],
                                AluOpType.mult, AluOpType.add,
                            )
                            den = sp.tile([128, 1], DT.float32, tag="den")
                            nc.scalar.activation(A[:, hb:hb + S], Bt[:, hb:hb + S],
                                                 AF.Exp, scale=SQD, accum_out=den[:])
                            rec = sp.tile([128, 1], DT.float32, tag="rec")
                            nc.vector.reciprocal(rec[:], den[:])
                            sc2 = sp.tile([128, 1], DT.float32, tag="sc2")
                            nc.vector.tensor_mul(sc2[:], rec[:], spike[:, ti:ti + 1])
                            pbt = pb_pool.tile([128, 1024], DT.bfloat16, tag="pbt")
                            nc.vector.tensor_scalar_mul(pbt[:, :S], A[:, hb:hb + S], sc2[:])
                            pbts[h] = pbt
                    for h in range(HL):
                        jc, hh = h // 2, h % 2
                        pT = pb_pool.tile([128, 1024], DT.bfloat16, tag="pT")
                        if ti < 6:
                            nc.sync.dma_start_transpose(
                                pT[:, :S].rearrange("p (b c) -> p b c", c=128),
                                pbts[h][:, :S],
                            )
                        else:
                            pstt = psT.tile([128, 1024], DT.bfloat16, tag="pstt")
                            for sj in range(ti + 1):
                                nc.tensor.transpose(
                                    pstt[:, sj * 128:(sj + 1) * 128],
                                    pbts[h][:, sj * 128:(sj + 1) * 128],
                                    idbf[:],
                                )
                            nc.scalar.copy(pT[:, :S], pstt[:, :S])
                        for sj in range(ti + 1):
                            nc.tensor.matmul(
                                psy[hh * 64:(hh + 1) * 64, jc * 128:(jc + 1) * 128],
                                vbf[:, sj * JD + h * D: sj * JD + (h + 1) * D],
                                pT[:, sj * 128:(sj + 1) * 128],
                                start=(sj == 0), stop=(sj == ti),
                                tile_position=(0, hh * 64),
                            )
                    # out projection for this t-tile
                    yT0 = sp.tile([128, 128], DT.bfloat16, tag="yT0")
                    yT1 = sp.tile([128, 128], DT.bfloat16, tag="yT1")
                    nc.vector.tensor_copy(yT0[:], psy[:, 0:128])
                    nc.vector.tensor_copy(yT1[:], psy[:, 128:256])
                    out_sb = po.tile([128, 1024], DT.float32, tag="outsb")
                    for oc in range(2):
                        pso = psA.tile([128, 1024], DT.float32, tag="ps")
                        for cc, yT_t in ((0, yT0), (1, yT1)):
                            nc.tensor.matmul(
                                pso[:, 0:512],
                                yT_t[:],
                                wobf[:, cc * C + oc * 512: cc * C + oc * 512 + 512],
                                start=(cc == 0), stop=(cc == 1),
                            )
                        nc.vector.tensor_copy(out_sb[:, oc * 512:(oc + 1) * 512], pso[:, 0:512])
                    if ti < 6:
                        nc.sync.dma_start(
                            out=partial1_d[ti * 128:(ti + 1) * 128, :], in_=out_sb[:]
                        )
                    else:
                        nc.sync.dma_start(
                            out=partial2_d[(ti - 6) * 128:(ti - 5) * 128, :], in_=out_sb[:]
                        )
                    # fire first reduce-scatter as soon as rows 0:768 are done
                    if ti == 5:
                        nc.gpsimd.collective_compute(
                            "ReduceScatter", mybir.AluOpType.add,
                            replica_groups=GROUPS,
                            ins=[partial1_d[:]],
                            outs=[rs1_d[:]],
                        )

                # ---- second reduce-scatter + bias + store ----
                nc.gpsimd.collective_compute(
                    "ReduceScatter", mybir.AluOpType.add,
                    replica_groups=GROUPS,
                    ins=[partial2_d[:]],
                    outs=[rs2_d[:]],
                )
                fin = po.tile([128, 1024], DT.float32, tag="fin")
                nc.sync.dma_start(out=fin[:], in_=rs1_d[0:128, :])
                nc.vector.tensor_add(fin[:], fin[:], bout_b[:])
                nc.sync.dma_start(out=out_e[0:128, :], in_=fin[:])
                fin2 = po.tile([128, 1024], DT.float32, tag="fin")
                nc.sync.dma_start(out=fin2[0:64, :], in_=rs1_d[128:192, :])
                nc.sync.dma_start(out=fin2[64:128, :], in_=rs2_d[:])
                nc.vector.tensor_add(fin2[:], fin2[:], bout_b[:])
                nc.sync.dma_start(out=out_e[128:256, :], in_=fin2[:])

    nc.finalize()
    return nc


_NC = None


def _get_nc():
    global _NC
    if _NC is None:
        _NC = build_nc()
    return _NC


def _shard_inputs(inputs):
    x = np.asarray(inputs["x"], np.float32)
    Wqkv = np.asarray(inputs["Wqkv"], np.float32)
    bqkv = np.asarray(inputs["bqkv"], np.float32)
    Wout = np.asarray(inputs["Wout"], np.float32)
    bout = np.asarray(inputs["bout"], np.float32)
    Wimp = np.asarray(inputs["Wimp"], np.float32)
    bimp = np.asarray(inputs["bimp"], np.float32)
    Walpha = np.asarray(inputs["Walpha"], np.float32)
    balpha = np.asarray(inputs["balpha"], np.float32)
    th = np.asarray(inputs["threshold"], np.float32)

    cmask = np.triu(np.full((128, 128), NEG, np.float32), 1)
    in_maps = []
    for core in range(N_CORES):
        b = core // 4
        hs = (core % 4) * HL
        sl = slice(hs * D, (hs + HL) * D)
        m = {
            "xT": np.ascontiguousarray(x[b].T),
            "wqT": np.ascontiguousarray(Wqkv[sl].T),
            "wkT": np.ascontiguousarray(Wqkv[C + hs * D: C + (hs + HL) * D].T),
            "wvT": np.ascontiguousarray(Wqkv[2 * C + hs * D: 2 * C + (hs + HL) * D].T),
            "bq_b": np.ascontiguousarray(np.broadcast_to(bqkv[sl], (128, JD))),
            "bk_b": np.ascontiguousarray(
                np.broadcast_to(bqkv[C + hs * D: C + (hs + HL) * D], (128, JD))),
            "bv_b": np.ascontiguousarray(
                np.broadcast_to(bqkv[2 * C + hs * D: 2 * C + (hs + HL) * D], (128, JD))),
            "wiaT": np.ascontiguousarray(
                np.concatenate([Wimp, Walpha[hs:hs + HL]], 0).T),
            "bia_b": np.ascontiguousarray(np.broadcast_to(
                np.concatenate([bimp, balpha[hs:hs + HL]]), (128, 5))),
            "woT": np.ascontiguousarray(Wout[:, sl].T),
            "bout_b": np.ascontiguousarray(np.broadcast_to(bout, (128, C))),
            "thneg_b": np.full((128, 1), -th[0], np.float32),
            "cmask": cmask,
        }
        in_maps.append(m)
    return in_maps


def kernel(**inputs):
    nc = _get_nc()
    in_maps = _shard_inputs(inputs)
    trace = os.environ.get("KERNEL_PROFILE", "") == "1"
    res = run_bass_kernel_spmd(
        nc, in_maps, core_ids=list(range(N_CORES)), trace=trace
    )
    KSTATS["exec_time_ns"] = res.exec_time_ns
    return _assemble({c: res.results[c] for c in range(N_CORES)})


def _assemble(results):
    out = np.zeros((B, T, C), np.float32)
    for core in range(N_CORES):
        b, r = core // 4, core % 4
        o = results[core]["out"]
        out[b, r * 192:(r + 1) * 192, :] = o[0:192]
        out[b, 768 + r * 64: 768 + (r + 1) * 64, :] = o[192:256]
    return out


# revision 20
# speedup vs baseline: 1.3261x; 1.0124x over previous
"""AdaptiveGeometryAttention distributed Bass kernel for 8 trn2 NeuronCores.

Sharding: data-parallel over B (2 groups of 4 cores), head-parallel over H
(4 heads per core). Each core computes its heads' attention and a partial
out-projection [T, C]; a ReduceScatter(add) over each 4-core group leaves
each core with a distinct 256-row shard of the final output, which the host
reassembles.

Self-contained: hardcodes all shapes; host side only shards/transposes
inputs and concatenates the output shards.
"""
import os
import sys

for _p in ("/opt/trn_rl_repo",):
    if _p not in sys.path:
        sys.path.append(_p)

import numpy as np
import concourse.bass as bass
import concourse.bacc as bacc
import concourse.mybir as mybir
from concourse import masks
from concourse.alu_op_type import AluOpType
from concourse.tile import TileContext
from concourse.bass_utils import run_bass_kernel_spmd

AF = mybir.ActivationFunctionType
DT = mybir.dt

B, T, C, H, D = 2, 1024, 1024, 16, 64
HL = 4                 # heads per core
JD = HL * D            # 256 local head dims
N_CORES = 8
GROUPS = [[0, 1, 2, 3], [4, 5, 6, 7]]
SQD = 0.125            # 1/sqrt(D)
NEG = -1.0e30

# dtype knobs
PROJ_F32R = True       # q/k/v/ia projection matmuls via float32r operands
NI_F32R = True         # neg_inner matmul via float32r operands

KSTATS = {}


def _f32r(ap):
    return ap.bitcast(DT.float32r)


def _mmdt(ap, use_f32r):
    return _f32r(ap) if use_f32r else ap


def build_nc():
    nc = bacc.Bacc("TRN2")

    # ---- I/O ----
    xT_e = nc.dram_tensor("xT", [C, T], DT.float32, kind="ExternalInput")
    DT_PROJ = DT.float32r if PROJ_F32R else DT.float32
    DT_NI = DT.float32r if NI_F32R else DT.float32
    wqT_e = nc.dram_tensor("wqT", [C, JD], DT_PROJ, kind="ExternalInput")
    wkT_e = nc.dram_tensor("wkT", [C, JD], DT_PROJ, kind="ExternalInput")
    wvT_e = nc.dram_tensor("wvT", [C, JD], DT_PROJ, kind="ExternalInput")
    bq_e = nc.dram_tensor("bq_b", [128, JD], DT.float32, kind="ExternalInput")
    bk_e = nc.dram_tensor("bk_b", [128, JD], DT.float32, kind="ExternalInput")
    bv_e = nc.dram_tensor("bv_b", [128, JD], DT.float32, kind="ExternalInput")
    wiaT_e = nc.dram_tensor("wiaT", [C, 5], DT.float32, kind="ExternalInput")
    bia_e = nc.dram_tensor("bia_b", [128, 5], DT.float32, kind="ExternalInput")
    woT_e = nc.dram_tensor("woT", [JD, C], DT.float32, kind="ExternalInput")
    bout_e = nc.dram_tensor("bout_b", [128, C], DT.float32, kind="ExternalInput")
    thneg_e = nc.dram_tensor("thneg_b", [128, 1], DT.float32, kind="ExternalInput")
    cmask_e = nc.dram_tensor("cmask", [128, 128], DT.float32, kind="ExternalInput")
    out_e = nc.dram_tensor("out", [T // 4, C], DT.float32, kind="ExternalOutput")

    partial1_d = nc.dram_tensor("partial1_d", [768, C], DT.float32)
    partial2_d = nc.dram_tensor("partial2_d", [256, C], DT.float32)
    rs1_d = nc.dram_tensor("rs1_d", [192, C], DT.float32)
    rs2_d = nc.dram_tensor("rs2_d", [64, C], DT.float32)

    with TileContext(nc) as tc:
        with (
            tc.tile_pool(name="const", bufs=1) as cpool,
            tc.tile_pool(name="mainp", bufs=1) as mp,
            tc.tile_pool(name="psA", bufs=3, space="PSUM") as psA,
            tc.tile_pool(name="psY", bufs=1, space="PSUM") as psY,
            tc.tile_pool(name="psT", bufs=1, space="PSUM") as psT,
        ):
            # ---- constants ----
            idf = cpool.tile([128, 128], DT.float32, tag="idf")
            masks.make_identity(nc, idf[:])
            idbf = cpool.tile([128, 128], DT.bfloat16, tag="idbf")
            masks.make_identity(nc, idbf[:])
            cmask = cpool.tile([128, 128], DT.float32, tag="cmask")
            nc.sync.dma_start(out=cmask[:], in_=cmask_e[:])
            bout_b = cpool.tile([128, C], DT.float32, tag="boutb")
            nc.sync.dma_start(out=bout_b[:], in_=bout_e[:])
            thneg = cpool.tile([128, 1], DT.float32, tag="thneg")
            nc.sync.dma_start(out=thneg[:], in_=thneg_e[:])
            negone = cpool.tile([128, 1], DT.float32, tag="negone")
            nc.vector.memset(negone[:], -1.0)

            # ---- persistent main tiles ----
            vbf = mp.tile([128, 8 * JD], DT.bfloat16, tag="vbf")
            qbT = [mp.tile([128, T], DT.bfloat16, tag=f"qbT{j}", name=f"qbT{j}") for j in range(2)]
            kbT = [mp.tile([128, T], DT.bfloat16, tag=f"kbT{j}", name=f"kbT{j}") for j in range(2)]
            qhT = [mp.tile([128, T], DT_NI, tag=f"qhT{j}", name=f"qhT{j}") for j in range(2)]
            khT = [mp.tile([128, T], DT_NI, tag=f"khT{j}", name=f"khT{j}") for j in range(2)]
            wobf = mp.tile([128, 2 * C], DT.bfloat16, tag="wobf")
            nalpha = mp.tile([128, 32], DT.float32, tag="nalpha")
            oma = mp.tile([128, 32], DT.float32, tag="oma")
            spike = mp.tile([128, 8], DT.float32, tag="spike")
            # row stats, col = side*32 + ti*4 + h
            rat = mp.tile([128, 64], DT.float32, tag="rat")    # sinh(n)/n
            gq = mp.tile([128, 32], DT.float32, tag="gq")      # cosh/ratio (q side)
            coshk = mp.tile([128, 32], DT.float32, tag="coshk")
            nrk = mp.tile([128, 32], DT.float32, tag="nrk")    # -ratio_k

            with tc.tile_pool(name="wpool", bufs=1) as wp:
                # ---- load weights/activations ----
                xT = wp.tile([128, 8 * T], DT.float32, tag="xT")
                for kc in range(8):
                    nc.sync.dma_start(
                        out=xT[:, kc * T:(kc + 1) * T],
                        in_=xT_e[kc * 128:(kc + 1) * 128, :],
                    )
                wq = wp.tile([128, 8 * JD], DT_PROJ, tag="wq")
                wk = wp.tile([128, 8 * JD], DT_PROJ, tag="wk")
                wv = wp.tile([128, 8 * JD], DT_PROJ, tag="wv")
                for w_t, w_e in ((wq, wqT_e), (wk, wkT_e), (wv, wvT_e)):
                    for kc in range(8):
                        nc.sync.dma_start(
                            out=w_t[:, kc * JD:(kc + 1) * JD],
                            in_=w_e[kc * 128:(kc + 1) * 128, :],
                        )
                if PROJ_F32R:
                    xTr = wp.tile([128, 8 * T], DT.float32r, tag="xTr")
                    nc.vector.tensor_copy(xTr[:], xT[:])
                else:
                    xTr = xT
                wia = wp.tile([128, 8 * 5], DT.float32, tag="wia")
                for kc in range(8):
                    nc.sync.dma_start(
                        out=wia[:, kc * 5:(kc + 1) * 5],
                        in_=wiaT_e[kc * 128:(kc + 1) * 128, :],
                    )
                wo = wp.tile([128, 2 * C], DT.float32, tag="wo")
                for cc in range(2):
                    nc.sync.dma_start(
                        out=wo[:, cc * C:(cc + 1) * C],
                        in_=woT_e[cc * 128:(cc + 1) * 128, :],
                    )
                bq_b = wp.tile([128, JD], DT.float32, tag="bqb")
                bk_b = wp.tile([128, JD], DT.float32, tag="bkb")
                bv_b = wp.tile([128, JD], DT.float32, tag="bvb")
                bia_b = wp.tile([128, 5], DT.float32, tag="biab")
                nc.sync.dma_start(out=bq_b[:], in_=bq_e[:])
                nc.sync.dma_start(out=bk_b[:], in_=bk_e[:])
                nc.sync.dma_start(out=bv_b[:], in_=bv_e[:])
                nc.sync.dma_start(out=bia_b[:], in_=bia_e[:])

                nc.vector.tensor_copy(wobf[:], wo[:])

                # ---- phase 1: projections (row layout [t, d']) ----
                qrow = wp.tile([128, 8 * JD], DT.float32, tag="qrow")
                krow = wp.tile([128, 8 * JD], DT.float32, tag="krow")
                ia_sb = wp.tile([128, 8 * 5], DT.float32, tag="iasb")

                # ia first so its sigmoids run before ln/exp table set loads
                for t8 in range(8):
                    ps = psA.tile([128, 1024], DT.float32, tag="ps")
                    for kc in range(8):
                        nc.tensor.matmul(
                            ps[:, 0:5],
                            xT[:, kc * T + t8 * 128: kc * T + t8 * 128 + 128],
                            wia[:, kc * 5:(kc + 1) * 5],
                            start=(kc == 0), stop=(kc == 7),
                        )
                    tmp5 = wp.tile([128, 5], DT.float32, tag="tmp5")
                    nc.vector.tensor_add(tmp5[:], ps[:, 0:5], bia_b[:])
                    nc.scalar.activation(ia_sb[:, t8 * 5:(t8 + 1) * 5], tmp5[:], AF.Sigmoid)

                for dst, w_t, b_t, outdt in (
                    (qrow, wq, bq_b, None),
                    (krow, wk, bk_b, None),
                    (vbf, wv, bv_b, DT.bfloat16),
                ):
                    for t8 in range(8):
                        ps = psA.tile([128, 1024], DT.float32, tag="ps")
                        for kc in range(8):
                            nc.tensor.matmul(
                                ps[:, 0:JD],
                                xTr[:, kc * T + t8 * 128: kc * T + t8 * 128 + 128],
                                w_t[:, kc * JD:(kc + 1) * JD],
                                start=(kc == 0), stop=(kc == 7),
                            )
                        nc.vector.tensor_add(
                            dst[:, t8 * JD:(t8 + 1) * JD], ps[:, 0:JD], b_t[:]
                        )

                # ---- spike / nalpha ----
                ia3 = ia_sb[:].rearrange("p (t f) -> p t f", f=5)
                # importance[t=0] := 0 (cmask) before threshold compare
                nc.vector.memset(ia_sb[0:1, 0:1], 0.0)
                imp8 = wp.tile([128, 8], DT.float32, tag="imp8")
                nc.vector.tensor_scalar(imp8[:], ia3[:, :, 0:1], thneg[:], None, AluOpType.add)
                sgn8 = wp.tile([128, 8], DT.float32, tag="sgn8")
                nc.scalar.activation(sgn8[:], imp8[:], AF.Sign)
                nc.vector.tensor_scalar_max(spike[:], sgn8[:], 0.0)
                nc.vector.tensor_scalar_mul(
                    nalpha[:].rearrange("p (t f) -> p t f", f=4), ia3[:, :, 1:5], -1.0
                )
                nc.vector.tensor_scalar(
                    oma[:].rearrange("p (t f) -> p t f", f=4), ia3[:, :, 1:5], -1.0,
                    1.0, AluOpType.mult, AluOpType.add,
                )

                # ---- phase 2: row stats + modified rows + transposes ----
                sqq = wp.tile([128, 8 * JD], DT.float32, tag="sqq")
                sqk = wp.tile([128, 8 * JD], DT.float32, tag="sqk")
                nc.scalar.activation(sqq[:], qrow[:], AF.Square)
                nc.scalar.activation(sqk[:], krow[:], AF.Square)
                n2 = wp.tile([128, 64], DT.float32, tag="n2")
                for side, sq_t in ((0, sqq), (1, sqk)):
                    for t8 in range(8):
                        for h in range(HL):
                            col = side * 32 + t8 * 4 + h
                            nc.vector.tensor_reduce(
                                n2[:, col:col + 1],
                                sq_t[:, t8 * JD + h * D + 1: t8 * JD + (h + 1) * D],
                                mybir.AxisListType.X, AluOpType.add,
                            )
                # n = max(exp(0.5*ln(n2)), 1e-7)
                lnn = wp.tile([128, 64], DT.float32, tag="lnn")
                nc.scalar.activation(lnn[:], n2[:], AF.Ln)
                nrm = wp.tile([128, 64], DT.float32, tag="nrm")
                nc.scalar.activation(nrm[:], lnn[:], AF.Exp, scale=0.5)
                nc.vector.tensor_scalar_max(nrm[:], nrm[:], 1e-7)
                e1 = wp.tile([128, 64], DT.float32, tag="e1")
                e2 = wp.tile([128, 64], DT.float32, tag="e2")
                nc.scalar.activation(e1[:], nrm[:], AF.Exp)
                nc.scalar.activation(e2[:], nrm[:], AF.Exp, scale=-1.0)
                csh = wp.tile([128, 64], DT.float32, tag="csh")
                nc.vector.tensor_add(csh[:], e1[:], e2[:])
                nc.vector.tensor_scalar_mul(csh[:], csh[:], 0.5)
                snh = wp.tile([128, 64], DT.float32, tag="snh")
                nc.vector.tensor_sub(snh[:], e1[:], e2[:])
                rcn = wp.tile([128, 64], DT.float32, tag="rcn")
                nc.vector.reciprocal(rcn[:], nrm[:])
                nc.vector.scalar_tensor_tensor(
                    rat[:], snh[:], 0.5, rcn[:], AluOpType.mult, AluOpType.mult
                )
                rrat = wp.tile([128, 64], DT.float32, tag="rrat")
                nc.vector.reciprocal(rrat[:], rat[:])
                nc.vector.tensor_mul(gq[:], csh[:, 0:32], rrat[:, 0:32])
                nc.vector.tensor_copy(coshk[:], csh[:, 32:64])
                nc.vector.tensor_scalar_mul(nrk[:], rat[:, 32:64], -1.0)

                # modified rows: qhat = qrow with col0 := gq; khat = -ratk*krow, col0 := coshk
                qhat = wp.tile([128, 8 * JD], DT.float32, tag="qhat")
                khat = wp.tile([128, 8 * JD], DT.float32, tag="khat")
                nc.vector.tensor_copy(qhat[:], qrow[:])
                for t8 in range(8):
                    for h in range(HL):
                        col = t8 * 4 + h
                        base = t8 * JD + h * D
                        nc.vector.tensor_copy(qhat[:, base:base + 1], gq[:, col:col + 1])
                        nc.vector.tensor_scalar_mul(
                            khat[:, base + 1: base + D],
                            krow[:, base + 1: base + D],
                            nrk[:, col:col + 1],
                        )
                        nc.vector.tensor_copy(khat[:, base:base + 1], coshk[:, col:col + 1])

                # transposes: row layout [t, c'] -> column layout [c', t]
                for src, dsts, dt_, ident in (
                    (qrow, qbT, DT.bfloat16, idf),
                    (krow, kbT, DT.bfloat16, idf),
                    (qhat, qhT, DT.float32, idf),
                    (khat, khT, DT.float32, idf),
                ):
                    for jc in range(2):
                        pst = psA.tile([128, 1024], DT.float32, tag="ps")
                        for t8 in range(8):
                            nc.tensor.transpose(
                                pst[:, t8 * 128:(t8 + 1) * 128],
                                src[:, t8 * JD + jc * 128: t8 * JD + (jc + 1) * 128],
                                ident[:],
                            )
                        nc.scalar.copy(dsts[jc][:], pst[:])

            # ---- phase 3: attention + out-projection ----
            with tc.tile_pool(name="pipe", bufs=2) as pp, \
                 tc.tile_pool(name="pipeb", bufs=3) as pb_pool, \
                 tc.tile_pool(name="pipeo", bufs=2) as po, \
                 tc.tile_pool(name="pipes", bufs=4) as sp:
                for ti in range(8):
                    S = (ti + 1) * 128
                    psy = psY.tile([128, 2 * 128], DT.float32, tag="psy")
                    pbts = {}
                    for pr in range(2):            # head pairs (0,1), (2,3)
                        A = pp.tile([128, 2048], DT.float32, tag="A", bufs=3)
                        Bt = pp.tile([128, 2048], DT.float32, tag="B", bufs=3)
                        Ct = pp.tile([128, 2048], DT.float32, tag="C", bufs=3)
                        for hh in range(2):
                            h = pr * 2 + hh
                            jc = h // 2
                            col = ti * 4 + h
                            se = psA.tile([128, 1024], DT.float32, tag="ps")
                            ni = psA.tile([128, 1024], DT.float32, tag="ps")
                            for c0 in range(0, S, 512):
                                n_sc = min(512, S - c0)
                                nc.tensor.matmul(
                                    se[:, c0:c0 + n_sc],
                                    qbT[jc][hh * 64:(hh + 1) * 64, ti * 128:(ti + 1) * 128],
                                    kbT[jc][hh * 64:(hh + 1) * 64, c0:c0 + n_sc],
                                    start=True, stop=True,
                                )
                                nc.tensor.matmul(
                                    ni[:, c0:c0 + n_sc],
                                    qhT[jc][hh * 64:(hh + 1) * 64, ti * 128:(ti + 1) * 128],
                                    khT[jc][hh * 64:(hh + 1) * 64, c0:c0 + n_sc],
                                    start=True, stop=True,
                                )
                            # A_h = (1-alpha)*se (+ cmask on diagonal block)
                            hb = hh * S
                            if ti > 0:
                                nc.vector.tensor_scalar(
                                    A[:, hb:hb + ti * 128], se[:, :ti * 128],
                                    oma[:, col:col + 1], None, AluOpType.mult,
                                )
                            nc.vector.scalar_tensor_tensor(
                                A[:, hb + ti * 128:hb + S], se[:, ti * 128:S],
                                oma[:, col:col + 1], cmask[:],
                                AluOpType.mult, AluOpType.add,
                            )
                            # B_h = max(ratio_q * ni_raw, 1+1e-7)
                            nc.vector.tensor_scalar(
                                Bt[:, hb:hb + S], ni[:, :S], rat[:, col:col + 1],
                                1.0 + 1e-7, AluOpType.mult, AluOpType.max,
                            )
                        W2 = 2 * S
                        # gpsimd's in-order queue blocks at the ti==5 collective,
                        # so tiles after it must not depend on gpsimd compute
                        eng = nc.gpsimd if ti < 6 else nc.vector
                        eng.tensor_mul(Ct[:, :W2], Bt[:, :W2], Bt[:, :W2])
                        nc.scalar.activation(Ct[:, :W2], Ct[:, :W2], AF.Ln, bias=negone[:])
                        nc.scalar.activation(Ct[:, :W2], Ct[:, :W2], AF.Exp, scale=0.5)
                        eng.tensor_add(Bt[:, :W2], Bt[:, :W2], Ct[:, :W2])
                        nc.scalar.activation(Bt[:, :W2], Bt[:, :W2], AF.Ln)
                        eng.tensor_mul(Bt[:, :W2], Bt[:, :W2], Bt[:, :W2])
                        for hh in range(2):
                            h = pr * 2 + hh
                            jc = h // 2
                            col = ti * 4 + h
                            hb = hh * S
                            # z = -alpha*d^2 + (1-alpha)*se   (in place into B)
                            nc.vector.scalar_tensor_tensor(
                                Bt[:, hb:hb + S], Bt[:, hb:hb + S],
                                nalpha[:, col:col + 1], A[:, hb:hb + S],
                                AluOpType.mult, AluOpType.add,
                            )
                            den = sp.tile([128, 1], DT.float32, tag="den")
                            nc.scalar.activation(A[:, hb:hb + S], Bt[:, hb:hb + S],
                                                 AF.Exp, scale=SQD, accum_out=den[:])
                            rec = sp.tile([128, 1], DT.float32, tag="rec")
                            nc.vector.reciprocal(rec[:], den[:])
                            sc2 = sp.tile([128, 1], DT.float32, tag="sc2")
                            nc.vector.tensor_mul(sc2[:], rec[:], spike[:, ti:ti + 1])
                            pbt = pb_pool.tile([128, 1024], DT.bfloat16, tag="pbt")
                            nc.vector.tensor_scalar_mul(pbt[:, :S], A[:, hb:hb + S], sc2[:])
                            pbts[h] = pbt
                    for h in range(HL):
                        jc, hh = h // 2, h % 2
                        pT = pb_pool.tile([128, 1024], DT.bfloat16, tag="pT")
                        if ti < 6:
                            nc.sync.dma_start_transpose(
                                pT[:, :S].rearrange("p (b c) -> p b c", c=128),
                                pbts[h][:, :S],
                            )
                        else:
                            pstt = psT.tile([128, 1024], DT.bfloat16, tag="pstt")
                            for sj in range(ti + 1):
                                nc.tensor.transpose(
                                    pstt[:, sj * 128:(sj + 1) * 128],
                                    pbts[h][:, sj * 128:(sj + 1) * 128],
                                    idbf[:],
                                )
                            nc.scalar.copy(pT[:, :S], pstt[:, :S])
                        for sj in range(ti + 1):
                            nc.tensor.matmul(
                                psy[hh * 64:(hh + 1) * 64, jc * 128:(jc + 1) * 128],
                                vbf[:, sj * JD + h * D: sj * JD + (h + 1) * D],
                                pT[:, sj * 128:(sj + 1) * 128],
                                start=(sj == 0), stop=(sj == ti),
                                tile_position=(0, hh * 64),
                            )
                    # out projection for this t-tile
                    yT0 = sp.tile([128, 128], DT.bfloat16, tag="yT0")
                    yT1 = sp.tile([128, 128], DT.bfloat16, tag="yT1")
                    nc.vector.tensor_copy(yT0[:], psy[:, 0:128])
                    nc.vector.tensor_copy(yT1[:], psy[:, 128:256])
                    out_sb = po.tile([128, 1024], DT.float32, tag="outsb")
                    for oc in range(2):
                        pso = psA.tile([128, 1024], DT.float32, tag="ps")
                        for cc, yT_t in ((0, yT0), (1, yT1)):
                            nc.tensor.matmul(
                                pso[:, 0:512],
                                yT_t[:],
                                wobf[:, cc * C + oc * 512: cc * C + oc * 512 + 512],
                                start=(cc == 0), stop=(cc == 1),
                            )
                        nc.vector.tensor_copy(out_sb[:, oc * 512:(oc + 1) * 512], pso[:, 0:512])
                    if ti < 6:
                        nc.sync.dma_start(
                            out=partial1_d[ti * 128:(ti + 1) * 128, :], in_=out_sb[:]
                        )
                    else:
                        nc.sync.dma_start(
                            out=partial2_d[(ti - 6) * 128:(ti - 5) * 128, :], in_=out_sb[:]
                        )
                    # fire first reduce-scatter as soon as rows 0:768 are done
                    if ti == 5:
                        nc.gpsimd.collective_compute(
                            "ReduceScatter", mybir.AluOpType.add,
                            replica_groups=GROUPS,
                            ins=[partial1_d[:]],
                            outs=[rs1_d[:]],
                        )

                # ---- second reduce-scatter + bias + store ----
                nc.gpsimd.collective_compute(
                    "ReduceScatter", mybir.AluOpType.add,
                    replica_groups=GROUPS,
                    ins=[partial2_d[:]],
                    outs=[rs2_d[:]],
                )
                fin = po.tile([128, 1024], DT.float32, tag="fin")
                nc.sync.dma_start(out=fin[:], in_=rs1_d[0:128, :])
                nc.vector.tensor_add(fin[:], fin[:], bout_b[:])
                nc.sync.dma_start(out=out_e[0:128, :], in_=fin[:])
                fin2 = po.tile([128, 1024], DT.float32, tag="fin")
                nc.sync.dma_start(out=fin2[0:64, :], in_=rs1_d[128:192, :])
                nc.sync.dma_start(out=fin2[64:128, :], in_=rs2_d[:])
                nc.vector.tensor_add(fin2[:], fin2[:], bout_b[:])
                nc.sync.dma_start(out=out_e[128:256, :], in_=fin2[:])

    nc.finalize()
    return nc


_NC = None


def _get_nc():
    global _NC
    if _NC is None:
        _NC = build_nc()
    return _NC


def _shard_inputs(inputs):
    x = np.asarray(inputs["x"], np.float32)
    Wqkv = np.asarray(inputs["Wqkv"], np.float32)
    bqkv = np.asarray(inputs["bqkv"], np.float32)
    Wout = np.asarray(inputs["Wout"], np.float32)
    bout = np.asarray(inputs["bout"], np.float32)
    Wimp = np.asarray(inputs["Wimp"], np.float32)
    bimp = np.asarray(inputs["bimp"], np.float32)
    Walpha = np.asarray(inputs["Walpha"], np.float32)
    balpha = np.asarray(inputs["balpha"], np.float32)
    th = np.asarray(inputs["threshold"], np.float32)

    cmask = np.triu(np.full((128, 128), NEG, np.float32), 1)
    in_maps = []
    for core in range(N_CORES):
        b = core // 4
        hs = (core % 4) * HL
        sl = slice(hs * D, (hs + HL) * D)
        m = {
            "xT": np.ascontiguousarray(x[b].T),
            "wqT": np.ascontiguousarray(Wqkv[sl].T),
            "wkT": np.ascontiguousarray(Wqkv[C + hs * D: C + (hs + HL) * D].T),
            "wvT": np.ascontiguousarray(Wqkv[2 * C + hs * D: 2 * C + (hs + HL) * D].T),
            "bq_b": np.ascontiguousarray(np.broadcast_to(bqkv[sl], (128, JD))),
            "bk_b": np.ascontiguousarray(
                np.broadcast_to(bqkv[C + hs * D: C + (hs + HL) * D], (128, JD))),
            "bv_b": np.ascontiguousarray(
                np.broadcast_to(bqkv[2 * C + hs * D: 2 * C + (hs + HL) * D], (128, JD))),
            "wiaT": np.ascontiguousarray(
                np.concatenate([Wimp, Walpha[hs:hs + HL]], 0).T),
            "bia_b": np.ascontiguousarray(np.broadcast_to(
                np.concatenate([bimp, balpha[hs:hs + HL]]), (128, 5))),
            "woT": np.ascontiguousarray(Wout[:, sl].T),
            "bout_b": np.ascontiguousarray(np.broadcast_to(bout, (128, C))),
            "thneg_b": np.full((128, 1), -th[0], np.float32),
            "cmask": cmask,
        }
        in_maps.append(m)
    return in_maps


def kernel(**inputs):
    nc = _get_nc()
    in_maps = _shard_inputs(inputs)
    trace = os.environ.get("KERNEL_PROFILE", "") == "1"
    res = run_bass_kernel_spmd(
        nc, in_maps, core_ids=list(range(N_CORES)), trace=trace
    )
    KSTATS["exec_time_ns"] = res.exec_time_ns
    return _assemble({c: res.results[c] for c in range(N_CORES)})


def _assemble(results):
    out = np.zeros((B, T, C), np.float32)
    for core in range(N_CORES):
        b, r = core // 4, core % 4
        o = results[core]["out"]
        out[b, r * 192:(r + 1) * 192, :] = o[0:192]
        out[b, 768 + r * 64: 768 + (r + 1) * 64, :] = o[192:256]
    return out


# revision 22
# speedup vs baseline: 1.3749x; 1.0368x over previous
"""AdaptiveGeometryAttention distributed Bass kernel for 8 trn2 NeuronCores.

Sharding: data-parallel over B (2 groups of 4 cores), head-parallel over H
(4 heads per core). Each core computes its heads' attention and a partial
out-projection [T, C]; a ReduceScatter(add) over each 4-core group leaves
each core with a distinct 256-row shard of the final output, which the host
reassembles.

Self-contained: hardcodes all shapes; host side only shards/transposes
inputs and concatenates the output shards.
"""
import os
import sys

for _p in ("/opt/trn_rl_repo",):
    if _p not in sys.path:
        sys.path.append(_p)

import numpy as np
import concourse.bass as bass
import concourse.bacc as bacc
import concourse.mybir as mybir
from concourse import masks
from concourse.alu_op_type import AluOpType
from concourse.tile import TileContext
from concourse.bass_utils import run_bass_kernel_spmd

AF = mybir.ActivationFunctionType
DT = mybir.dt

B, T, C, H, D = 2, 1024, 1024, 16, 64
HL = 4                 # heads per core
JD = HL * D            # 256 local head dims
N_CORES = 8
GROUPS = [[0, 1, 2, 3], [4, 5, 6, 7]]
SQD = 0.125            # 1/sqrt(D)
NEG = -1.0e30

# dtype knobs
PROJ_F32R = True       # q/k/v/ia projection matmuls via float32r operands
NI_F32R = True         # neg_inner matmul via float32r operands

KSTATS = {}


def _f32r(ap):
    return ap.bitcast(DT.float32r)


def _mmdt(ap, use_f32r):
    return _f32r(ap) if use_f32r else ap


def build_nc():
    nc = bacc.Bacc("TRN2")

    # ---- I/O ----
    xT_e = nc.dram_tensor("xT", [C, T], DT.float32, kind="ExternalInput")
    DT_PROJ = DT.float32r if PROJ_F32R else DT.float32
    DT_NI = DT.float32r if NI_F32R else DT.float32
    wqT_e = nc.dram_tensor("wqT", [C, JD], DT_PROJ, kind="ExternalInput")
    wkT_e = nc.dram_tensor("wkT", [C, JD], DT_PROJ, kind="ExternalInput")
    wvT_e = nc.dram_tensor("wvT", [C, JD], DT_PROJ, kind="ExternalInput")
    bq_e = nc.dram_tensor("bq_b", [128, JD], DT.float32, kind="ExternalInput")
    bk_e = nc.dram_tensor("bk_b", [128, JD], DT.float32, kind="ExternalInput")
    bv_e = nc.dram_tensor("bv_b", [128, JD], DT.float32, kind="ExternalInput")
    wiaT_e = nc.dram_tensor("wiaT", [C, 5], DT.float32, kind="ExternalInput")
    bia_e = nc.dram_tensor("bia_b", [128, 5], DT.float32, kind="ExternalInput")
    woT_e = nc.dram_tensor("woT", [JD, C], DT.float32, kind="ExternalInput")
    bout_e = nc.dram_tensor("bout_b", [128, C], DT.float32, kind="ExternalInput")
    thneg_e = nc.dram_tensor("thneg_b", [128, 1], DT.float32, kind="ExternalInput")
    cmask_e = nc.dram_tensor("cmask", [128, 128], DT.float32, kind="ExternalInput")
    out_e = nc.dram_tensor("out", [T // 4, C], DT.float32, kind="ExternalOutput")

    partial_d = nc.dram_tensor("partial_d", [T, C], DT.bfloat16)
    rs_out_d = nc.dram_tensor("rs_out_d", [T // 4, C], DT.bfloat16)

    with TileContext(nc) as tc:
        with (
            tc.tile_pool(name="const", bufs=1) as cpool,
            tc.tile_pool(name="mainp", bufs=1) as mp,
            tc.tile_pool(name="psA", bufs=3, space="PSUM") as psA,
            tc.tile_pool(name="psY", bufs=2, space="PSUM") as psY,
        ):
            # ---- constants ----
            idf = cpool.tile([128, 128], DT.float32, tag="idf")
            masks.make_identity(nc, idf[:])
            idbf = cpool.tile([128, 128], DT.bfloat16, tag="idbf")
            masks.make_identity(nc, idbf[:])
            cmask = cpool.tile([128, 128], DT.float32, tag="cmask")
            nc.sync.dma_start(out=cmask[:], in_=cmask_e[:])
            bout_b = cpool.tile([128, C], DT.float32, tag="boutb")
            nc.sync.dma_start(out=bout_b[:], in_=bout_e[:])
            thneg = cpool.tile([128, 1], DT.float32, tag="thneg")
            nc.sync.dma_start(out=thneg[:], in_=thneg_e[:])
            negone = cpool.tile([128, 1], DT.float32, tag="negone")
            nc.vector.memset(negone[:], -1.0)

            # ---- persistent main tiles ----
            vbf = mp.tile([128, 8 * JD], DT.bfloat16, tag="vbf")
            qbT = [mp.tile([128, T], DT.bfloat16, tag=f"qbT{j}", name=f"qbT{j}") for j in range(2)]
            kbT = [mp.tile([128, T], DT.bfloat16, tag=f"kbT{j}", name=f"kbT{j}") for j in range(2)]
            qhT = [mp.tile([128, T], DT_NI, tag=f"qhT{j}", name=f"qhT{j}") for j in range(2)]
            khT = [mp.tile([128, T], DT_NI, tag=f"khT{j}", name=f"khT{j}") for j in range(2)]
            wobf = mp.tile([128, 2 * C], DT.bfloat16, tag="wobf")
            nalpha = mp.tile([128, 32], DT.float32, tag="nalpha")
            oma = mp.tile([128, 32], DT.float32, tag="oma")
            spike = mp.tile([128, 8], DT.float32, tag="spike")
            # row stats, col = side*32 + ti*4 + h
            rat = mp.tile([128, 64], DT.float32, tag="rat")    # sinh(n)/n
            gq = mp.tile([128, 32], DT.float32, tag="gq")      # cosh/ratio (q side)
            coshk = mp.tile([128, 32], DT.float32, tag="coshk")
            nrk = mp.tile([128, 32], DT.float32, tag="nrk")    # -ratio_k

            with tc.tile_pool(name="wpool", bufs=1) as wp:
                # ---- load weights/activations ----
                xT = wp.tile([128, 8 * T], DT.float32, tag="xT")
                for kc in range(8):
                    nc.sync.dma_start(
                        out=xT[:, kc * T:(kc + 1) * T],
                        in_=xT_e[kc * 128:(kc + 1) * 128, :],
                    )
                wq = wp.tile([128, 8 * JD], DT_PROJ, tag="wq")
                wk = wp.tile([128, 8 * JD], DT_PROJ, tag="wk")
                wv = wp.tile([128, 8 * JD], DT_PROJ, tag="wv")
                for w_t, w_e in ((wq, wqT_e), (wk, wkT_e), (wv, wvT_e)):
                    for kc in range(8):
                        nc.sync.dma_start(
                            out=w_t[:, kc * JD:(kc + 1) * JD],
                            in_=w_e[kc * 128:(kc + 1) * 128, :],
                        )
                if PROJ_F32R:
                    xTr = wp.tile([128, 8 * T], DT.float32r, tag="xTr")
                    nc.vector.tensor_copy(xTr[:], xT[:])
                else:
                    xTr = xT
                wia = wp.tile([128, 8 * 5], DT.float32, tag="wia")
                for kc in range(8):
                    nc.sync.dma_start(
                        out=wia[:, kc * 5:(kc + 1) * 5],
                        in_=wiaT_e[kc * 128:(kc + 1) * 128, :],
                    )
                wo = wp.tile([128, 2 * C], DT.float32, tag="wo")
                for cc in range(2):
                    nc.sync.dma_start(
                        out=wo[:, cc * C:(cc + 1) * C],
                        in_=woT_e[cc * 128:(cc + 1) * 128, :],
                    )
                bq_b = wp.tile([128, JD], DT.float32, tag="bqb")
                bk_b = wp.tile([128, JD], DT.float32, tag="bkb")
                bv_b = wp.tile([128, JD], DT.float32, tag="bvb")
                bia_b = wp.tile([128, 5], DT.float32, tag="biab")
                nc.sync.dma_start(out=bq_b[:], in_=bq_e[:])
                nc.sync.dma_start(out=bk_b[:], in_=bk_e[:])
                nc.sync.dma_start(out=bv_b[:], in_=bv_e[:])
                nc.sync.dma_start(out=bia_b[:], in_=bia_e[:])

                nc.vector.tensor_copy(wobf[:], wo[:])

                # ---- phase 1: projections (row layout [t, d']) ----
                qrow = wp.tile([128, 8 * JD], DT.float32, tag="qrow")
                krow = wp.tile([128, 8 * JD], DT.float32, tag="krow")
                ia_sb = wp.tile([128, 8 * 5], DT.float32, tag="iasb")

                # ia first so its sigmoids run before ln/exp table set loads
                for t8 in range(8):
                    ps = psA.tile([128, 1024], DT.float32, tag="ps")
                    for kc in range(8):
                        nc.tensor.matmul(
                            ps[:, 0:5],
                            xT[:, kc * T + t8 * 128: kc * T + t8 * 128 + 128],
                            wia[:, kc * 5:(kc + 1) * 5],
                            start=(kc == 0), stop=(kc == 7),
                        )
                    tmp5 = wp.tile([128, 5], DT.float32, tag="tmp5")
                    nc.vector.tensor_add(tmp5[:], ps[:, 0:5], bia_b[:])
                    nc.scalar.activation(ia_sb[:, t8 * 5:(t8 + 1) * 5], tmp5[:], AF.Sigmoid)

                for dst, w_t, b_t, outdt in (
                    (qrow, wq, bq_b, None),
                    (krow, wk, bk_b, None),
                    (vbf, wv, bv_b, DT.bfloat16),
                ):
                    for t8 in range(8):
                        ps = psA.tile([128, 1024], DT.float32, tag="ps")
                        for kc in range(8):
                            nc.tensor.matmul(
                                ps[:, 0:JD],
                                xTr[:, kc * T + t8 * 128: kc * T + t8 * 128 + 128],
                                w_t[:, kc * JD:(kc + 1) * JD],
                                start=(kc == 0), stop=(kc == 7),
                            )
                        nc.vector.tensor_add(
                            dst[:, t8 * JD:(t8 + 1) * JD], ps[:, 0:JD], b_t[:]
                        )

                # ---- spike / nalpha ----
                ia3 = ia_sb[:].rearrange("p (t f) -> p t f", f=5)
                # importance[t=0] := 0 (cmask) before threshold compare
                nc.vector.memset(ia_sb[0:1, 0:1], 0.0)
                imp8 = wp.tile([128, 8], DT.float32, tag="imp8")
                nc.vector.tensor_scalar(imp8[:], ia3[:, :, 0:1], thneg[:], None, AluOpType.add)
                sgn8 = wp.tile([128, 8], DT.float32, tag="sgn8")
                nc.scalar.activation(sgn8[:], imp8[:], AF.Sign)
                nc.vector.tensor_scalar_max(spike[:], sgn8[:], 0.0)
                nc.vector.tensor_scalar_mul(
                    nalpha[:].rearrange("p (t f) -> p t f", f=4), ia3[:, :, 1:5], -1.0
                )
                nc.vector.tensor_scalar(
                    oma[:].rearrange("p (t f) -> p t f", f=4), ia3[:, :, 1:5], -1.0,
                    1.0, AluOpType.mult, AluOpType.add,
                )

                # ---- phase 2: row stats + modified rows + transposes ----
                sqq = wp.tile([128, 8 * JD], DT.float32, tag="sqq")
                sqk = wp.tile([128, 8 * JD], DT.float32, tag="sqk")
                nc.scalar.activation(sqq[:], qrow[:], AF.Square)
                nc.scalar.activation(sqk[:], krow[:], AF.Square)
                n2 = wp.tile([128, 64], DT.float32, tag="n2")
                for side, sq_t in ((0, sqq), (1, sqk)):
                    for t8 in range(8):
                        for h in range(HL):
                            col = side * 32 + t8 * 4 + h
                            nc.vector.tensor_reduce(
                                n2[:, col:col + 1],
                                sq_t[:, t8 * JD + h * D + 1: t8 * JD + (h + 1) * D],
                                mybir.AxisListType.X, AluOpType.add,
                            )
                # n = max(exp(0.5*ln(n2)), 1e-7)
                lnn = wp.tile([128, 64], DT.float32, tag="lnn")
                nc.scalar.activation(lnn[:], n2[:], AF.Ln)
                nrm = wp.tile([128, 64], DT.float32, tag="nrm")
                nc.scalar.activation(nrm[:], lnn[:], AF.Exp, scale=0.5)
                nc.vector.tensor_scalar_max(nrm[:], nrm[:], 1e-7)
                e1 = wp.tile([128, 64], DT.float32, tag="e1")
                e2 = wp.tile([128, 64], DT.float32, tag="e2")
                nc.scalar.activation(e1[:], nrm[:], AF.Exp)
                nc.scalar.activation(e2[:], nrm[:], AF.Exp, scale=-1.0)
                csh = wp.tile([128, 64], DT.float32, tag="csh")
                nc.vector.tensor_add(csh[:], e1[:], e2[:])
                nc.vector.tensor_scalar_mul(csh[:], csh[:], 0.5)
                snh = wp.tile([128, 64], DT.float32, tag="snh")
                nc.vector.tensor_sub(snh[:], e1[:], e2[:])
                rcn = wp.tile([128, 64], DT.float32, tag="rcn")
                nc.vector.reciprocal(rcn[:], nrm[:])
                nc.vector.scalar_tensor_tensor(
                    rat[:], snh[:], 0.5, rcn[:], AluOpType.mult, AluOpType.mult
                )
                rrat = wp.tile([128, 64], DT.float32, tag="rrat")
                nc.vector.reciprocal(rrat[:], rat[:])
                nc.vector.tensor_mul(gq[:], csh[:, 0:32], rrat[:, 0:32])
                nc.vector.tensor_copy(coshk[:], csh[:, 32:64])
                nc.vector.tensor_scalar_mul(nrk[:], rat[:, 32:64], -1.0)

                # modified rows: qhat = qrow with col0 := gq; khat = -ratk*krow, col0 := coshk
                qhat = wp.tile([128, 8 * JD], DT.float32, tag="qhat")
                khat = wp.tile([128, 8 * JD], DT.float32, tag="khat")
                nc.vector.tensor_copy(qhat[:], qrow[:])
                for t8 in range(8):
                    for h in range(HL):
                        col = t8 * 4 + h
                        base = t8 * JD + h * D
                        nc.vector.tensor_copy(qhat[:, base:base + 1], gq[:, col:col + 1])
                        nc.vector.tensor_scalar_mul(
                            khat[:, base + 1: base + D],
                            krow[:, base + 1: base + D],
                            nrk[:, col:col + 1],
                        )
                        nc.vector.tensor_copy(khat[:, base:base + 1], coshk[:, col:col + 1])

                # transposes: row layout [t, c'] -> column layout [c', t]
                for src, dsts, dt_, ident in (
                    (qrow, qbT, DT.bfloat16, idf),
                    (krow, kbT, DT.bfloat16, idf),
                    (qhat, qhT, DT.float32, idf),
                    (khat, khT, DT.float32, idf),
                ):
                    for jc in range(2):
                        pst = psA.tile([128, 1024], DT.float32, tag="ps")
                        for t8 in range(8):
                            nc.tensor.transpose(
                                pst[:, t8 * 128:(t8 + 1) * 128],
                                src[:, t8 * JD + jc * 128: t8 * JD + (jc + 1) * 128],
                                ident[:],
                            )
                        nc.scalar.copy(dsts[jc][:], pst[:])

            # ---- phase 3: attention + out-projection ----
            with tc.tile_pool(name="pipe", bufs=2) as pp, \
                 tc.tile_pool(name="pipeb", bufs=3) as pb_pool, \
                 tc.tile_pool(name="pipeo", bufs=2) as po, \
                 tc.tile_pool(name="pipes", bufs=4) as sp:
                for ti in range(8):
                    S = (ti + 1) * 128
                    psy = psY.tile([128, 2 * 128], DT.float32, tag="psy")
                    pbts = {}
                    for pr in range(2):            # head pairs (0,1), (2,3)
                        A = pp.tile([128, 2048], DT.float32, tag="A", bufs=3)
                        Bt = pp.tile([128, 2048], DT.float32, tag="B", bufs=3)
                        Ct = pp.tile([128, 2048], DT.float32, tag="C", bufs=3)
                        for hh in range(2):
                            h = pr * 2 + hh
                            jc = h // 2
                            col = ti * 4 + h
                            se = psA.tile([128, 1024], DT.float32, tag="ps")
                            ni = psA.tile([128, 1024], DT.float32, tag="ps")
                            for c0 in range(0, S, 512):
                                n_sc = min(512, S - c0)
                                nc.tensor.matmul(
                                    se[:, c0:c0 + n_sc],
                                    qbT[jc][hh * 64:(hh + 1) * 64, ti * 128:(ti + 1) * 128],
                                    kbT[jc][hh * 64:(hh + 1) * 64, c0:c0 + n_sc],
                                    start=True, stop=True,
                                )
                                nc.tensor.matmul(
                                    ni[:, c0:c0 + n_sc],
                                    qhT[jc][hh * 64:(hh + 1) * 64, ti * 128:(ti + 1) * 128],
                                    khT[jc][hh * 64:(hh + 1) * 64, c0:c0 + n_sc],
                                    start=True, stop=True,
                                )
                            # A_h = (1-alpha)*se (+ cmask on diagonal block)
                            hb = hh * S
                            if ti > 0:
                                nc.vector.tensor_scalar(
                                    A[:, hb:hb + ti * 128], se[:, :ti * 128],
                                    oma[:, col:col + 1], None, AluOpType.mult,
                                )
                            nc.vector.scalar_tensor_tensor(
                                A[:, hb + ti * 128:hb + S], se[:, ti * 128:S],
                                oma[:, col:col + 1], cmask[:],
                                AluOpType.mult, AluOpType.add,
                            )
                            # B_h = max(ratio_q * ni_raw, 1+1e-7)
                            nc.vector.tensor_scalar(
                                Bt[:, hb:hb + S], ni[:, :S], rat[:, col:col + 1],
                                1.0 + 1e-7, AluOpType.mult, AluOpType.max,
                            )
                        W2 = 2 * S
                        eng = nc.gpsimd
                        eng.tensor_mul(Ct[:, :W2], Bt[:, :W2], Bt[:, :W2])
                        nc.scalar.activation(Ct[:, :W2], Ct[:, :W2], AF.Ln, bias=negone[:])
                        nc.scalar.activation(Ct[:, :W2], Ct[:, :W2], AF.Exp, scale=0.5)
                        eng.tensor_add(Bt[:, :W2], Bt[:, :W2], Ct[:, :W2])
                        nc.scalar.activation(Bt[:, :W2], Bt[:, :W2], AF.Ln)
                        eng.tensor_mul(Bt[:, :W2], Bt[:, :W2], Bt[:, :W2])
                        for hh in range(2):
                            h = pr * 2 + hh
                            jc = h // 2
                            col = ti * 4 + h
                            hb = hh * S
                            # z = -alpha*d^2 + (1-alpha)*se   (in place into B)
                            nc.vector.scalar_tensor_tensor(
                                Bt[:, hb:hb + S], Bt[:, hb:hb + S],
                                nalpha[:, col:col + 1], A[:, hb:hb + S],
                                AluOpType.mult, AluOpType.add,
                            )
                            den = sp.tile([128, 1], DT.float32, tag="den")
                            nc.scalar.activation(A[:, hb:hb + S], Bt[:, hb:hb + S],
                                                 AF.Exp, scale=SQD, accum_out=den[:])
                            rec = sp.tile([128, 1], DT.float32, tag="rec")
                            nc.vector.reciprocal(rec[:], den[:])
                            sc2 = sp.tile([128, 1], DT.float32, tag="sc2")
                            nc.vector.tensor_mul(sc2[:], rec[:], spike[:, ti:ti + 1])
                            pbt = pb_pool.tile([128, 1024], DT.bfloat16, tag="pbt")
                            nc.vector.tensor_scalar_mul(pbt[:, :S], A[:, hb:hb + S], sc2[:])
                            pbts[h] = pbt
                    for h in range(HL):
                        jc, hh = h // 2, h % 2
                        pT = pb_pool.tile([128, 1024], DT.bfloat16, tag="pT")
                        nc.sync.dma_start_transpose(
                            pT[:, :S].rearrange("p (b c) -> p b c", c=128),
                            pbts[h][:, :S],
                        )
                        for sj in range(ti + 1):
                            nc.tensor.matmul(
                                psy[hh * 64:(hh + 1) * 64, jc * 128:(jc + 1) * 128],
                                vbf[:, sj * JD + h * D: sj * JD + (h + 1) * D],
                                pT[:, sj * 128:(sj + 1) * 128],
                                start=(sj == 0), stop=(sj == ti),
                                tile_position=(0, hh * 64),
                            )
                    # out projection for this t-tile (bf16 partial)
                    yT0 = sp.tile([128, 128], DT.bfloat16, tag="yT0")
                    yT1 = sp.tile([128, 128], DT.bfloat16, tag="yT1")
                    nc.vector.tensor_copy(yT0[:], psy[:, 0:128])
                    nc.vector.tensor_copy(yT1[:], psy[:, 128:256])
                    out_sb = po.tile([128, 1024], DT.bfloat16, tag="outsb")
                    for oc in range(2):
                        pso = psA.tile([128, 1024], DT.float32, tag="ps")
                        for cc, yT_t in ((0, yT0), (1, yT1)):
                            nc.tensor.matmul(
                                pso[:, 0:512],
                                yT_t[:],
                                wobf[:, cc * C + oc * 512: cc * C + oc * 512 + 512],
                                start=(cc == 0), stop=(cc == 1),
                            )
                        nc.vector.tensor_copy(out_sb[:, oc * 512:(oc + 1) * 512], pso[:, 0:512])
                    nc.sync.dma_start(
                        out=partial_d[ti * 128:(ti + 1) * 128, :], in_=out_sb[:]
                    )

                # ---- single bf16 reduce-scatter at the end ----
                nc.gpsimd.collective_compute(
                    "ReduceScatter", mybir.AluOpType.add,
                    replica_groups=GROUPS,
                    ins=[partial_d[:]],
                    outs=[rs_out_d[:]],
                )
                for r2 in range(2):
                    finb = po.tile([128, 1024], DT.bfloat16, tag="finb")
                    nc.sync.dma_start(out=finb[:], in_=rs_out_d[r2 * 128:(r2 + 1) * 128, :])
                    fin = po.tile([128, 1024], DT.float32, tag="fin")
                    nc.vector.tensor_add(fin[:], finb[:], bout_b[:])
                    nc.sync.dma_start(out=out_e[r2 * 128:(r2 + 1) * 128, :], in_=fin[:])

    nc.finalize()
    return nc


_NC = None


def _get_nc():
    global _NC
    if _NC is None:
        _NC = build_nc()
    return _NC


def _shard_inputs(inputs):
    x = np.asarray(inputs["x"], np.float32)
    Wqkv = np.asarray(inputs["Wqkv"], np.float32)
    bqkv = np.asarray(inputs["bqkv"], np.float32)
    Wout = np.asarray(inputs["Wout"], np.float32)
    bout = np.asarray(inputs["bout"], np.float32)
    Wimp = np.asarray(inputs["Wimp"], np.float32)
    bimp = np.asarray(inputs["bimp"], np.float32)
    Walpha = np.asarray(inputs["Walpha"], np.float32)
    balpha = np.asarray(inputs["balpha"], np.float32)
    th = np.asarray(inputs["threshold"], np.float32)

    cmask = np.triu(np.full((128, 128), NEG, np.float32), 1)
    in_maps = []
    for core in range(N_CORES):
        b = core // 4
        hs = (core % 4) * HL
        sl = slice(hs * D, (hs + HL) * D)
        m = {
            "xT": np.ascontiguousarray(x[b].T),
            "wqT": np.ascontiguousarray(Wqkv[sl].T),
            "wkT": np.ascontiguousarray(Wqkv[C + hs * D: C + (hs + HL) * D].T),
            "wvT": np.ascontiguousarray(Wqkv[2 * C + hs * D: 2 * C + (hs + HL) * D].T),
            "bq_b": np.ascontiguousarray(np.broadcast_to(bqkv[sl], (128, JD))),
            "bk_b": np.ascontiguousarray(
                np.broadcast_to(bqkv[C + hs * D: C + (hs + HL) * D], (128, JD))),
            "bv_b": np.ascontiguousarray(
                np.broadcast_to(bqkv[2 * C + hs * D: 2 * C + (hs + HL) * D], (128, JD))),
            "wiaT": np.ascontiguousarray(
                np.concatenate([Wimp, Walpha[hs:hs + HL]], 0).T),
            "bia_b": np.ascontiguousarray(np.broadcast_to(
                np.concatenate([bimp, balpha[hs:hs + HL]]), (128, 5))),
            "woT": np.ascontiguousarray(Wout[:, sl].T),
            "bout_b": np.ascontiguousarray(np.broadcast_to(bout, (128, C))),
            "thneg_b": np.full((128, 1), -th[0], np.float32),
            "cmask": cmask,
        }
        in_maps.append(m)
    return in_maps


def kernel(**inputs):
    nc = _get_nc()
    in_maps = _shard_inputs(inputs)
    trace = os.environ.get("KERNEL_PROFILE", "") == "1"
    res = run_bass_kernel_spmd(
        nc, in_maps, core_ids=list(range(N_CORES)), trace=trace
    )
    KSTATS["exec_time_ns"] = res.exec_time_ns
    return _assemble({c: res.results[c] for c in range(N_CORES)})


def _assemble(results):
    out = np.zeros((B, T, C), np.float32)
    for core in range(N_CORES):
        b, r = core // 4, core % 4
        out[b, r * 256:(r + 1) * 256, :] = results[core]["out"]
    return out


# revision 23
# speedup vs baseline: 1.4690x; 1.0684x over previous
"""AdaptiveGeometryAttention distributed Bass kernel for 8 trn2 NeuronCores.

Sharding: data-parallel over B (2 groups of 4 cores), head-parallel over H
(4 heads per core). Each core computes its heads' attention and a partial
out-projection [T, C]; a ReduceScatter(add) over each 4-core group leaves
each core with a distinct 256-row shard of the final output, which the host
reassembles.

Self-contained: hardcodes all shapes; host side only shards/transposes
inputs and concatenates the output shards.
"""
import os
import sys

for _p in ("/opt/trn_rl_repo",):
    if _p not in sys.path:
        sys.path.append(_p)

import numpy as np
import concourse.bass as bass
import concourse.bacc as bacc
import concourse.mybir as mybir
from concourse import masks
from concourse.alu_op_type import AluOpType
from concourse.tile import TileContext
from concourse.bass_utils import run_bass_kernel_spmd

AF = mybir.ActivationFunctionType
DT = mybir.dt

B, T, C, H, D = 2, 1024, 1024, 16, 64
HL = 4                 # heads per core
JD = HL * D            # 256 local head dims
N_CORES = 8
GROUPS = [[0, 1, 2, 3], [4, 5, 6, 7]]
SQD = 0.125            # 1/sqrt(D)
NEG = -1.0e30

# dtype knobs
PROJ_F32R = True       # q/k/v/ia projection matmuls via float32r operands
NI_F32R = True         # neg_inner matmul via float32r operands

KSTATS = {}

# The act-table-load placement pass picks the FIRST set containing each
# activation function, so alternating Ln/Exp thrashes between the
# single-function sets (~50 table reloads per kernel). Strip ln/exp from
# those sets so the combined natural_log_exp_and_others set is chosen.
_orig_get_tables = bacc.get_activation_tables


def _patched_get_tables(arch):
    t = _orig_get_tables(arch)
    for nm in ("exp_and_others", "natural_log", "exp_and_friends"):
        if nm in t:
            t[nm] = t[nm] - {AF.Exp, AF.Ln}
    return t


bacc.get_activation_tables = _patched_get_tables


def _f32r(ap):
    return ap.bitcast(DT.float32r)


def _mmdt(ap, use_f32r):
    return _f32r(ap) if use_f32r else ap


def build_nc():
    nc = bacc.Bacc("TRN2")

    # ---- I/O ----
    xT_e = nc.dram_tensor("xT", [C, T], DT.float32, kind="ExternalInput")
    DT_PROJ = DT.float32r if PROJ_F32R else DT.float32
    DT_NI = DT.float32r if NI_F32R else DT.float32
    wqT_e = nc.dram_tensor("wqT", [C, JD], DT_PROJ, kind="ExternalInput")
    wkT_e = nc.dram_tensor("wkT", [C, JD], DT_PROJ, kind="ExternalInput")
    wvT_e = nc.dram_tensor("wvT", [C, JD], DT_PROJ, kind="ExternalInput")
    bq_e = nc.dram_tensor("bq_b", [128, JD], DT.float32, kind="ExternalInput")
    bk_e = nc.dram_tensor("bk_b", [128, JD], DT.float32, kind="ExternalInput")
    bv_e = nc.dram_tensor("bv_b", [128, JD], DT.float32, kind="ExternalInput")
    wiaT_e = nc.dram_tensor("wiaT", [C, 5], DT.float32, kind="ExternalInput")
    bia_e = nc.dram_tensor("bia_b", [128, 5], DT.float32, kind="ExternalInput")
    woT_e = nc.dram_tensor("woT", [JD, C], DT.float32, kind="ExternalInput")
    bout_e = nc.dram_tensor("bout_b", [128, C], DT.float32, kind="ExternalInput")
    thneg_e = nc.dram_tensor("thneg_b", [128, 1], DT.float32, kind="ExternalInput")
    cmask_e = nc.dram_tensor("cmask", [128, 128], DT.float32, kind="ExternalInput")
    out_e = nc.dram_tensor("out", [T // 4, C], DT.float32, kind="ExternalOutput")

    partial_d = nc.dram_tensor("partial_d", [T, C], DT.bfloat16)
    rs_out_d = nc.dram_tensor("rs_out_d", [T // 4, C], DT.bfloat16)

    with TileContext(nc) as tc:
        with (
            tc.tile_pool(name="const", bufs=1) as cpool,
            tc.tile_pool(name="mainp", bufs=1) as mp,
            tc.tile_pool(name="psA", bufs=3, space="PSUM") as psA,
            tc.tile_pool(name="psY", bufs=2, space="PSUM") as psY,
        ):
            # ---- constants ----
            idf = cpool.tile([128, 128], DT.float32, tag="idf")
            masks.make_identity(nc, idf[:])
            idbf = cpool.tile([128, 128], DT.bfloat16, tag="idbf")
            masks.make_identity(nc, idbf[:])
            cmask = cpool.tile([128, 128], DT.float32, tag="cmask")
            nc.sync.dma_start(out=cmask[:], in_=cmask_e[:])
            bout_b = cpool.tile([128, C], DT.float32, tag="boutb")
            nc.sync.dma_start(out=bout_b[:], in_=bout_e[:])
            thneg = cpool.tile([128, 1], DT.float32, tag="thneg")
            nc.sync.dma_start(out=thneg[:], in_=thneg_e[:])
            negone = cpool.tile([128, 1], DT.float32, tag="negone")
            nc.vector.memset(negone[:], -1.0)

            # ---- persistent main tiles ----
            vbf = mp.tile([128, 8 * JD], DT.bfloat16, tag="vbf")
            qbT = [mp.tile([128, T], DT.bfloat16, tag=f"qbT{j}", name=f"qbT{j}") for j in range(2)]
            kbT = [mp.tile([128, T], DT.bfloat16, tag=f"kbT{j}", name=f"kbT{j}") for j in range(2)]
            qhT = [mp.tile([128, T], DT_NI, tag=f"qhT{j}", name=f"qhT{j}") for j in range(2)]
            khT = [mp.tile([128, T], DT_NI, tag=f"khT{j}", name=f"khT{j}") for j in range(2)]
            wobf = mp.tile([128, 2 * C], DT.bfloat16, tag="wobf")
            nalpha = mp.tile([128, 32], DT.float32, tag="nalpha")
            oma = mp.tile([128, 32], DT.float32, tag="oma")
            spike = mp.tile([128, 8], DT.float32, tag="spike")
            # row stats, col = side*32 + ti*4 + h
            rat = mp.tile([128, 64], DT.float32, tag="rat")    # sinh(n)/n
            gq = mp.tile([128, 32], DT.float32, tag="gq")      # cosh/ratio (q side)
            coshk = mp.tile([128, 32], DT.float32, tag="coshk")
            nrk = mp.tile([128, 32], DT.float32, tag="nrk")    # -ratio_k

            with tc.tile_pool(name="wpool", bufs=1) as wp:
                # ---- load weights/activations ----
                xT = wp.tile([128, 8 * T], DT.float32, tag="xT")
                for kc in range(8):
                    nc.sync.dma_start(
                        out=xT[:, kc * T:(kc + 1) * T],
                        in_=xT_e[kc * 128:(kc + 1) * 128, :],
                    )
                wq = wp.tile([128, 8 * JD], DT_PROJ, tag="wq")
                wk = wp.tile([128, 8 * JD], DT_PROJ, tag="wk")
                wv = wp.tile([128, 8 * JD], DT_PROJ, tag="wv")
                for w_t, w_e in ((wq, wqT_e), (wk, wkT_e), (wv, wvT_e)):
                    for kc in range(8):
                        nc.sync.dma_start(
                            out=w_t[:, kc * JD:(kc + 1) * JD],
                            in_=w_e[kc * 128:(kc + 1) * 128, :],
                        )
                if PROJ_F32R:
                    xTr = wp.tile([128, 8 * T], DT.float32r, tag="xTr")
                    nc.vector.tensor_copy(xTr[:], xT[:])
                else:
                    xTr = xT
                wia = wp.tile([128, 8 * 5], DT.float32, tag="wia")
                for kc in range(8):
                    nc.sync.dma_start(
                        out=wia[:, kc * 5:(kc + 1) * 5],
                        in_=wiaT_e[kc * 128:(kc + 1) * 128, :],
                    )
                wo = wp.tile([128, 2 * C], DT.float32, tag="wo")
                for cc in range(2):
                    nc.sync.dma_start(
                        out=wo[:, cc * C:(cc + 1) * C],
                        in_=woT_e[cc * 128:(cc + 1) * 128, :],
                    )
                bq_b = wp.tile([128, JD], DT.float32, tag="bqb")
                bk_b = wp.tile([128, JD], DT.float32, tag="bkb")
                bv_b = wp.tile([128, JD], DT.float32, tag="bvb")
                bia_b = wp.tile([128, 5], DT.float32, tag="biab")
                nc.sync.dma_start(out=bq_b[:], in_=bq_e[:])
                nc.sync.dma_start(out=bk_b[:], in_=bk_e[:])
                nc.sync.dma_start(out=bv_b[:], in_=bv_e[:])
                nc.sync.dma_start(out=bia_b[:], in_=bia_e[:])

                nc.vector.tensor_copy(wobf[:], wo[:])

                # ---- phase 1: projections (row layout [t, d']) ----
                qrow = wp.tile([128, 8 * JD], DT.float32, tag="qrow")
                krow = wp.tile([128, 8 * JD], DT.float32, tag="krow")
                ia_sb = wp.tile([128, 8 * 5], DT.float32, tag="iasb")

                # ia first so its sigmoids run before ln/exp table set loads
                for t8 in range(8):
                    ps = psA.tile([128, 1024], DT.float32, tag="ps")
                    for kc in range(8):
                        nc.tensor.matmul(
                            ps[:, 0:5],
                            xT[:, kc * T + t8 * 128: kc * T + t8 * 128 + 128],
                            wia[:, kc * 5:(kc + 1) * 5],
                            start=(kc == 0), stop=(kc == 7),
                        )
                    tmp5 = wp.tile([128, 5], DT.float32, tag="tmp5")
                    nc.vector.tensor_add(tmp5[:], ps[:, 0:5], bia_b[:])
                    nc.scalar.activation(ia_sb[:, t8 * 5:(t8 + 1) * 5], tmp5[:], AF.Sigmoid)

                for dst, w_t, b_t, outdt in (
                    (qrow, wq, bq_b, None),
                    (krow, wk, bk_b, None),
                    (vbf, wv, bv_b, DT.bfloat16),
                ):
                    for t8 in range(8):
                        ps = psA.tile([128, 1024], DT.float32, tag="ps")
                        for kc in range(8):
                            nc.tensor.matmul(
                                ps[:, 0:JD],
                                xTr[:, kc * T + t8 * 128: kc * T + t8 * 128 + 128],
                                w_t[:, kc * JD:(kc + 1) * JD],
                                start=(kc == 0), stop=(kc == 7),
                            )
                        nc.vector.tensor_add(
                            dst[:, t8 * JD:(t8 + 1) * JD], ps[:, 0:JD], b_t[:]
                        )

                # ---- spike / nalpha ----
                ia3 = ia_sb[:].rearrange("p (t f) -> p t f", f=5)
                # importance[t=0] := 0 (cmask) before threshold compare
                nc.vector.memset(ia_sb[0:1, 0:1], 0.0)
                imp8 = wp.tile([128, 8], DT.float32, tag="imp8")
                nc.vector.tensor_scalar(imp8[:], ia3[:, :, 0:1], thneg[:], None, AluOpType.add)
                sgn8 = wp.tile([128, 8], DT.float32, tag="sgn8")
                nc.scalar.activation(sgn8[:], imp8[:], AF.Sign)
                nc.vector.tensor_scalar_max(spike[:], sgn8[:], 0.0)
                nc.vector.tensor_scalar_mul(
                    nalpha[:].rearrange("p (t f) -> p t f", f=4), ia3[:, :, 1:5], -1.0
                )
                nc.vector.tensor_scalar(
                    oma[:].rearrange("p (t f) -> p t f", f=4), ia3[:, :, 1:5], -1.0,
                    1.0, AluOpType.mult, AluOpType.add,
                )

                # ---- phase 2: row stats + modified rows + transposes ----
                sqq = wp.tile([128, 8 * JD], DT.float32, tag="sqq")
                sqk = wp.tile([128, 8 * JD], DT.float32, tag="sqk")
                nc.scalar.activation(sqq[:], qrow[:], AF.Square)
                nc.scalar.activation(sqk[:], krow[:], AF.Square)
                n2 = wp.tile([128, 64], DT.float32, tag="n2")
                for side, sq_t in ((0, sqq), (1, sqk)):
                    for t8 in range(8):
                        for h in range(HL):
                            col = side * 32 + t8 * 4 + h
                            nc.vector.tensor_reduce(
                                n2[:, col:col + 1],
                                sq_t[:, t8 * JD + h * D + 1: t8 * JD + (h + 1) * D],
                                mybir.AxisListType.X, AluOpType.add,
                            )
                # n = max(exp(0.5*ln(n2)), 1e-7)
                lnn = wp.tile([128, 64], DT.float32, tag="lnn")
                nc.scalar.activation(lnn[:], n2[:], AF.Ln)
                nrm = wp.tile([128, 64], DT.float32, tag="nrm")
                nc.scalar.activation(nrm[:], lnn[:], AF.Exp, scale=0.5)
                nc.vector.tensor_scalar_max(nrm[:], nrm[:], 1e-7)
                e1 = wp.tile([128, 64], DT.float32, tag="e1")
                e2 = wp.tile([128, 64], DT.float32, tag="e2")
                nc.scalar.activation(e1[:], nrm[:], AF.Exp)
                nc.scalar.activation(e2[:], nrm[:], AF.Exp, scale=-1.0)
                csh = wp.tile([128, 64], DT.float32, tag="csh")
                nc.vector.tensor_add(csh[:], e1[:], e2[:])
                nc.vector.tensor_scalar_mul(csh[:], csh[:], 0.5)
                snh = wp.tile([128, 64], DT.float32, tag="snh")
                nc.vector.tensor_sub(snh[:], e1[:], e2[:])
                rcn = wp.tile([128, 64], DT.float32, tag="rcn")
                nc.vector.reciprocal(rcn[:], nrm[:])
                nc.vector.scalar_tensor_tensor(
                    rat[:], snh[:], 0.5, rcn[:], AluOpType.mult, AluOpType.mult
                )
                rrat = wp.tile([128, 64], DT.float32, tag="rrat")
                nc.vector.reciprocal(rrat[:], rat[:])
                nc.vector.tensor_mul(gq[:], csh[:, 0:32], rrat[:, 0:32])
                nc.vector.tensor_copy(coshk[:], csh[:, 32:64])
                nc.vector.tensor_scalar_mul(nrk[:], rat[:, 32:64], -1.0)

                # modified rows: qhat = qrow with col0 := gq; khat = -ratk*krow, col0 := coshk
                qhat = wp.tile([128, 8 * JD], DT.float32, tag="qhat")
                khat = wp.tile([128, 8 * JD], DT.float32, tag="khat")
                nc.vector.tensor_copy(qhat[:], qrow[:])
                for t8 in range(8):
                    for h in range(HL):
                        col = t8 * 4 + h
                        base = t8 * JD + h * D
                        nc.vector.tensor_copy(qhat[:, base:base + 1], gq[:, col:col + 1])
                        nc.vector.tensor_scalar_mul(
                            khat[:, base + 1: base + D],
                            krow[:, base + 1: base + D],
                            nrk[:, col:col + 1],
                        )
                        nc.vector.tensor_copy(khat[:, base:base + 1], coshk[:, col:col + 1])

                # transposes: row layout [t, c'] -> column layout [c', t]
                for src, dsts, dt_, ident in (
                    (qrow, qbT, DT.bfloat16, idf),
                    (krow, kbT, DT.bfloat16, idf),
                    (qhat, qhT, DT.float32, idf),
                    (khat, khT, DT.float32, idf),
                ):
                    for jc in range(2):
                        pst = psA.tile([128, 1024], DT.float32, tag="ps")
                        for t8 in range(8):
                            nc.tensor.transpose(
                                pst[:, t8 * 128:(t8 + 1) * 128],
                                src[:, t8 * JD + jc * 128: t8 * JD + (jc + 1) * 128],
                                ident[:],
                            )
                        nc.scalar.copy(dsts[jc][:], pst[:])

            # ---- phase 3: attention + out-projection ----
            with tc.tile_pool(name="pipe", bufs=2) as pp, \
                 tc.tile_pool(name="pipeb", bufs=3) as pb_pool, \
                 tc.tile_pool(name="pipeo", bufs=2) as po, \
                 tc.tile_pool(name="pipes", bufs=4) as sp:
                for ti in range(8):
                    S = (ti + 1) * 128
                    psy = psY.tile([128, 2 * 128], DT.float32, tag="psy")
                    pbts = {}
                    for pr in range(2):            # head pairs (0,1), (2,3)
                        A = pp.tile([128, 2048], DT.float32, tag="A", bufs=3)
                        Bt = pp.tile([128, 2048], DT.float32, tag="B", bufs=3)
                        Ct = pp.tile([128, 2048], DT.float32, tag="C", bufs=3)
                        for hh in range(2):
                            h = pr * 2 + hh
                            jc = h // 2
                            col = ti * 4 + h
                            se = psA.tile([128, 1024], DT.float32, tag="ps")
                            ni = psA.tile([128, 1024], DT.float32, tag="ps")
                            for c0 in range(0, S, 512):
                                n_sc = min(512, S - c0)
                                nc.tensor.matmul(
                                    se[:, c0:c0 + n_sc],
                                    qbT[jc][hh * 64:(hh + 1) * 64, ti * 128:(ti + 1) * 128],
                                    kbT[jc][hh * 64:(hh + 1) * 64, c0:c0 + n_sc],
                                    start=True, stop=True,
                                )
                                nc.tensor.matmul(
                                    ni[:, c0:c0 + n_sc],
                                    qhT[jc][hh * 64:(hh + 1) * 64, ti * 128:(ti + 1) * 128],
                                    khT[jc][hh * 64:(hh + 1) * 64, c0:c0 + n_sc],
                                    start=True, stop=True,
                                )
                            # A_h = (1-alpha)*se (+ cmask on diagonal block)
                            hb = hh * S
                            if ti > 0:
                                nc.vector.tensor_scalar(
                                    A[:, hb:hb + ti * 128], se[:, :ti * 128],
                                    oma[:, col:col + 1], None, AluOpType.mult,
                                )
                            nc.vector.scalar_tensor_tensor(
                                A[:, hb + ti * 128:hb + S], se[:, ti * 128:S],
                                oma[:, col:col + 1], cmask[:],
                                AluOpType.mult, AluOpType.add,
                            )
                            # B_h = max(ratio_q * ni_raw, 1+1e-7)
                            nc.vector.tensor_scalar(
                                Bt[:, hb:hb + S], ni[:, :S], rat[:, col:col + 1],
                                1.0 + 1e-7, AluOpType.mult, AluOpType.max,
                            )
                        W2 = 2 * S
                        eng = nc.gpsimd
                        eng.tensor_mul(Ct[:, :W2], Bt[:, :W2], Bt[:, :W2])
                        nc.scalar.activation(Ct[:, :W2], Ct[:, :W2], AF.Ln, bias=negone[:])
                        nc.scalar.activation(Ct[:, :W2], Ct[:, :W2], AF.Exp, scale=0.5)
                        eng.tensor_add(Bt[:, :W2], Bt[:, :W2], Ct[:, :W2])
                        nc.scalar.activation(Bt[:, :W2], Bt[:, :W2], AF.Ln)
                        eng.tensor_mul(Bt[:, :W2], Bt[:, :W2], Bt[:, :W2])
                        for hh in range(2):
                            h = pr * 2 + hh
                            jc = h // 2
                            col = ti * 4 + h
                            hb = hh * S
                            # z = -alpha*d^2 + (1-alpha)*se   (in place into B)
                            nc.vector.scalar_tensor_tensor(
                                Bt[:, hb:hb + S], Bt[:, hb:hb + S],
                                nalpha[:, col:col + 1], A[:, hb:hb + S],
                                AluOpType.mult, AluOpType.add,
                            )
                            den = sp.tile([128, 1], DT.float32, tag="den")
                            nc.scalar.activation(A[:, hb:hb + S], Bt[:, hb:hb + S],
                                                 AF.Exp, scale=SQD, accum_out=den[:])
                            rec = sp.tile([128, 1], DT.float32, tag="rec")
                            nc.vector.reciprocal(rec[:], den[:])
                            sc2 = sp.tile([128, 1], DT.float32, tag="sc2")
                            nc.vector.tensor_mul(sc2[:], rec[:], spike[:, ti:ti + 1])
                            pbt = pb_pool.tile([128, 1024], DT.bfloat16, tag="pbt")
                            nc.vector.tensor_scalar_mul(pbt[:, :S], A[:, hb:hb + S], sc2[:])
                            pbts[h] = pbt
                    for h in range(HL):
                        jc, hh = h // 2, h % 2
                        pT = pb_pool.tile([128, 1024], DT.bfloat16, tag="pT")
                        nc.sync.dma_start_transpose(
                            pT[:, :S].rearrange("p (b c) -> p b c", c=128),
                            pbts[h][:, :S],
                        )
                        for sj in range(ti + 1):
                            nc.tensor.matmul(
                                psy[hh * 64:(hh + 1) * 64, jc * 128:(jc + 1) * 128],
                                vbf[:, sj * JD + h * D: sj * JD + (h + 1) * D],
                                pT[:, sj * 128:(sj + 1) * 128],
                                start=(sj == 0), stop=(sj == ti),
                                tile_position=(0, hh * 64),
                            )
                    # out projection for this t-tile (bf16 partial)
                    yT0 = sp.tile([128, 128], DT.bfloat16, tag="yT0")
                    yT1 = sp.tile([128, 128], DT.bfloat16, tag="yT1")
                    nc.vector.tensor_copy(yT0[:], psy[:, 0:128])
                    nc.vector.tensor_copy(yT1[:], psy[:, 128:256])
                    out_sb = po.tile([128, 1024], DT.bfloat16, tag="outsb")
                    for oc in range(2):
                        pso = psA.tile([128, 1024], DT.float32, tag="ps")
                        for cc, yT_t in ((0, yT0), (1, yT1)):
                            nc.tensor.matmul(
                                pso[:, 0:512],
                                yT_t[:],
                                wobf[:, cc * C + oc * 512: cc * C + oc * 512 + 512],
                                start=(cc == 0), stop=(cc == 1),
                            )
                        nc.vector.tensor_copy(out_sb[:, oc * 512:(oc + 1) * 512], pso[:, 0:512])
                    nc.sync.dma_start(
                        out=partial_d[ti * 128:(ti + 1) * 128, :], in_=out_sb[:]
                    )

                # ---- single bf16 reduce-scatter at the end ----
                nc.gpsimd.collective_compute(
                    "ReduceScatter", mybir.AluOpType.add,
                    replica_groups=GROUPS,
                    ins=[partial_d[:]],
                    outs=[rs_out_d[:]],
                )
                for r2 in range(2):
                    finb = po.tile([128, 1024], DT.bfloat16, tag="finb")
                    nc.sync.dma_start(out=finb[:], in_=rs_out_d[r2 * 128:(r2 + 1) * 128, :])
                    fin = po.tile([128, 1024], DT.float32, tag="fin")
                    nc.vector.tensor_add(fin[:], finb[:], bout_b[:])
                    nc.sync.dma_start(out=out_e[r2 * 128:(r2 + 1) * 128, :], in_=fin[:])

    nc.finalize()
    return nc


_NC = None


def _get_nc():
    global _NC
    if _NC is None:
        _NC = build_nc()
    return _NC


def _shard_inputs(inputs):
    x = np.asarray(inputs["x"], np.float32)
    Wqkv = np.asarray(inputs["Wqkv"], np.float32)
    bqkv = np.asarray(inputs["bqkv"], np.float32)
    Wout = np.asarray(inputs["Wout"], np.float32)
    bout = np.asarray(inputs["bout"], np.float32)
    Wimp = np.asarray(inputs["Wimp"], np.float32)
    bimp = np.asarray(inputs["bimp"], np.float32)
    Walpha = np.asarray(inputs["Walpha"], np.float32)
    balpha = np.asarray(inputs["balpha"], np.float32)
    th = np.asarray(inputs["threshold"], np.float32)

    cmask = np.triu(np.full((128, 128), NEG, np.float32), 1)
    in_maps = []
    for core in range(N_CORES):
        b = core // 4
        hs = (core % 4) * HL
        sl = slice(hs * D, (hs + HL) * D)
        m = {
            "xT": np.ascontiguousarray(x[b].T),
            "wqT": np.ascontiguousarray(Wqkv[sl].T),
            "wkT": np.ascontiguousarray(Wqkv[C + hs * D: C + (hs + HL) * D].T),
            "wvT": np.ascontiguousarray(Wqkv[2 * C + hs * D: 2 * C + (hs + HL) * D].T),
            "bq_b": np.ascontiguousarray(np.broadcast_to(bqkv[sl], (128, JD))),
            "bk_b": np.ascontiguousarray(
                np.broadcast_to(bqkv[C + hs * D: C + (hs + HL) * D], (128, JD))),
            "bv_b": np.ascontiguousarray(
                np.broadcast_to(bqkv[2 * C + hs * D: 2 * C + (hs + HL) * D], (128, JD))),
            "wiaT": np.ascontiguousarray(
                np.concatenate([Wimp, Walpha[hs:hs + HL]], 0).T),
            "bia_b": np.ascontiguousarray(np.broadcast_to(
                np.concatenate([bimp, balpha[hs:hs + HL]]), (128, 5))),
            "woT": np.ascontiguousarray(Wout[:, sl].T),
            "bout_b": np.ascontiguousarray(np.broadcast_to(bout, (128, C))),
            "thneg_b": np.full((128, 1), -th[0], np.float32),
            "cmask": cmask,
        }
        in_maps.append(m)
    return in_maps


def kernel(**inputs):
    nc = _get_nc()
    in_maps = _shard_inputs(inputs)
    trace = os.environ.get("KERNEL_PROFILE", "") == "1"
    res = run_bass_kernel_spmd(
        nc, in_maps, core_ids=list(range(N_CORES)), trace=trace
    )
    KSTATS["exec_time_ns"] = res.exec_time_ns
    return _assemble({c: res.results[c] for c in range(N_CORES)})


def _assemble(results):
    out = np.zeros((B, T, C), np.float32)
    for core in range(N_CORES):
        b, r = core // 4, core % 4
        out[b, r * 256:(r + 1) * 256, :] = results[core]["out"]
    return out
